# revision 63
# baseline (speedup 1.0000x reference)
"""Batch graph attention (GAT-style) Trainium2 kernel.

Problem: B=8, N=2048, F=64, FH=64, H=4.
  feats = X @ W[h]                         [B,H,N,FH]
  scores[n,m] = leaky_relu(s_self[n] + s_neigh[m], 0.2)
  P = softmax(scores + (1-A)*NEG_BIG, axis=m)
  out = relu(concat_h(P @ feats + b))

Sharding: batch b -> core b (8 cores, data parallel).

Per-core algorithm (neighbor index m on SBUF partitions):

  exp(leaky(x)) == max(e^x, e^{0.2x})  (slope<1); dropping the per-column
  factor e^{s_self[n]} (softmax columns are scale invariant) leaves

      Phat[m,n] = A^T[m,n] * max(e1[m], e2[m] * g[n])

  with e1=0.5*exp(s_neigh), e2=0.5*exp(0.2*s_neigh), g=exp(-0.8*s_self).
  Per (h,k-tile): u = (g_bc * e2) max e1 (DVE ts, 4x) and p = u * A^T
  (DVE tt, 2x).  Aggregation uses the TRANSPOSED matmul orientation:
  p-chunk [128m x 128n] is the PE stationary, G = [feats+b | 1] the
  65-col moving operand, accumulating agg[n, o] (+den at o=64) in PSUM
  per (head, ntile).  That makes den a per-partition column, so finals
  are: den cols -> SBUF (Act), one DVE reciprocal per head, and a
  per-ntile Act Relu(scale=1/den) straight out of PSUM.  Output leaves
  in natural [H, N, FH] orientation (host concatenates heads).

  A^T comes from fp32 A's fp16 bit-pair structure: fp16 view of fp32 1.0
  is [0x0000 | 0x3F80] = [0 | 1.875].  An xbar DMA transpose of 128 fp16
  columns starting at an ODD offset lands the 1.875*A values on EVEN
  output partitions (zeros on odd); the aligned window starting 128 later
  lands its values on ODD partitions.  A merge (DVE add, Pool add, or a
  partition-strided SBUF-to-SBUF DMA) produces dense 1.875*A^T with rows
  in the fixed interleave pi(p) = p/2 (p even) | 64+(p-1)/2 (p odd).
  The 1.875 cancels in the softmax; the pi permutation is absorbed by
  building XT16's columns pi-permuted, so G rows / e-vectors line up.
  g (an n-indexed row) is un-permuted during its PE transpose with the
  inverse identity.
"""

import numpy as np

B, N, F, FH, H = 8, 2048, 64, 64, 4
P = 128           # SBUF partitions
NT = N // P       # 16 m-tiles / n-tiles
C = 512           # chunk used for feats matmuls
NCH = N // C      # 4 chunks
GW = 66           # G row stride (64 feats + 1 ones + 1 pad)
LN_HALF = -0.6931471805599453

_CACHE = {}

# tuning knobs (read at build time)
KNOBS = {
    "tt_bufs": 4,         # xbar staging tile buffers
    "u_bufs": 13,
    "u_ahead": 10,         # emit u-ops this many seq steps ahead (needs u_bufs)
    "p_bufs": 5,
    "outp_bufs": 1,
    "lead": 2,            # merge lead (in k) ahead of consumption
    "pool_la": 4,         # lookahead (in seq steps) for pool-assigned ops
    "merge_dma_k": 16,    # merges with k >= this go via DMA
    "merge_pool": 0,      # first k merges on Pool (rest DVE)
    "prologue_merges": 5, # merges emitted before the first k-group
    "p_pool_1": 3,        # phase-1: every n-th (h,k) p-op on Pool
    "p_pool_2": 5,        # phase-2: every n-th (h,k) p-op on Pool
    "u_pool_1": 10,        # phase-1: every n-th (h,k) u-op on Pool
    "u_pool_2": 7,        # phase-2: every n-th (h,k) u-op on Pool
    "pool2_split": 0,     # independent per-sub-phase pool strides below
    "p_pool_2b": 3,
    "u_pool_2b": 4,
    "gbc_dma": True,      # g broadcast via DRAM bounce (else Pool)
    "gbc_pool_heads": (0, 1, 2),  # heads whose g broadcast goes on Pool
    "gbc_pe_heads": (),   # heads whose g broadcast goes via PE ones-matmul
    "fin_act": True,      # final relu/scale on Act (else DVE)
    "split_relu_at": 6,   # last head: relus with t >= this go on DVE
    "setup2_k": 14,        # phase-1 k at which head-2 setup is emitted
    "setup3_k": 15,       # phase-1 k at which head-3 setup is emitted
    "pe_warmup": 0,       # junk PE transposes for p-state ramp (0=off)
    "early_pairs": 0,     # merge tiles whose transposes precede X/W loads
    "half_merge_k": 2,    # first k merges done in column halves
    "quarter_merge_k": 0, # first k merges done in column quarters
    "merge_split_pool": 0,  # merges k in [2,2+n): lo half DVE, hi half Pool
    "prefill": 0,         # head-2 k-groups pre-built during phase-1 tail
    "act_split": 0,       # leading columns of u computed on Act (Prelu+Exp)
}

# agg PSUM bank grouping: ntiles per PSUM tile (65 f32 cols each; 7*65*4B
# = 1820B fits one 2KB bank, so slices never cross a bank boundary)
AGG_GROUPS = (7, 7, 2)


def _build():
    import concourse.bacc as bacc
    import concourse.tile as tile
    import concourse.mybir as mybir
    from concourse.mybir import AluOpType as op, ActivationFunctionType as act

    f32 = mybir.dt.float32
    fp16 = mybir.dt.float16
    i32 = mybir.dt.int32

    nc = bacc.Bacc(
        "TRN2",
        target_bir_lowering=False,
        debug=False,
        enable_asserts=False,
        num_devices=8,
    )

    A_d = nc.dram_tensor("A", [N, N], f32, kind="ExternalInput").ap()
    X_d = nc.dram_tensor("X", [N, F], f32, kind="ExternalInput").ap()
    W_d = nc.dram_tensor("W", [H, F, FH], f32, kind="ExternalInput").ap()
    b_d = nc.dram_tensor("b", [H, FH], f32, kind="ExternalInput").ap()
    # WA[h] = [W[h] @ a_self[h] | W[h] @ a_neigh[h]]  (tiny, host-precomputed)
    wa_d = nc.dram_tensor("WA", [H, F, 2], f32, kind="ExternalInput").ap()
    OUT_d = nc.dram_tensor("OUT", [H, N, FH], fp16, kind="ExternalOutput").ap()
    GROW_d = nc.dram_tensor("GROW", [H, N], fp16, kind="Internal").ap()
    SSROW_d = nc.dram_tensor("SSROW", [H, N], fp16, kind="Internal").ap()

    with tile.TileContext(nc) as tc:
        with (
            tc.tile_pool(name="const", bufs=1) as const,
            tc.tile_pool(name="big", bufs=1) as big,
            tc.tile_pool(name="stream", bufs=3) as stream,
            tc.tile_pool(name="head", bufs=2) as head,
            tc.tile_pool(name="outp", bufs=KNOBS["outp_bufs"]) as outp,
            tc.tile_pool(name="psu", bufs=2, space="PSUM") as psu,
            tc.tile_pool(name="psagg", bufs=2, space="PSUM") as psagg,
        ):
            # ---- constants --------------------------------------------
            iota_i = stream.tile([P, P], i32, tag="cst", bufs=2)
            nc.gpsimd.iota(iota_i[:], pattern=[[1, P]], base=0, channel_multiplier=0)
            pidx_i = stream.tile([P, 1], i32, tag="cst1", bufs=1)
            nc.gpsimd.iota(pidx_i[:], pattern=[[0, 1]], base=0, channel_multiplier=1)
            pidx_f = const.tile([P, 1], f32)
            nc.vector.tensor_copy(pidx_f[:], pidx_i[:])
            iota_f = stream.tile([P, P], f32, tag="cstf", bufs=2)
            nc.vector.tensor_copy(iota_f[:], iota_i[:])
            # pi sequence [0,64,1,65,...]: ident_pi[p,c]=1 iff p==pi(c)
            iopi_i = stream.tile([P, P], i32, tag="cst", bufs=2)
            nc.gpsimd.iota(iopi_i[:], pattern=[[1, 64], [64, 2]], base=0,
                           channel_multiplier=0)
            iopi_f = stream.tile([P, P], f32, tag="cstf", bufs=2)
            nc.vector.tensor_copy(iopi_f[:], iopi_i[:])
            ident_pi = const.tile([P, P], fp16)
            nc.vector.tensor_scalar(ident_pi[:], iopi_f[:], pidx_f[:], None,
                                    op.is_equal)
            # pinv sequence [0,2,..126,1,3,..127]: ident_pinv[p,c]=1 iff p==pinv(c)
            iopv_i = stream.tile([P, P], i32, tag="cst", bufs=2)
            nc.gpsimd.iota(iopv_i[:], pattern=[[1, 2], [2, 64]], base=0,
                           channel_multiplier=0)
            iopv_f = stream.tile([P, P], f32, tag="cstf", bufs=2)
            nc.vector.tensor_copy(iopv_f[:], iopv_i[:])
            ident_pinv = const.tile([P, P], fp16)
            nc.vector.tensor_scalar(ident_pinv[:], iopv_f[:], pidx_f[:], None,
                                    op.is_equal)

            lnhalf = const.tile([P, 1], f32)
            nc.vector.memset(lnhalf[:], LN_HALF)
            alpha02 = const.tile([P, 1], f32)
            nc.vector.memset(alpha02[:], 0.2)
            ones1 = const.tile([1, P], fp16)
            nc.vector.memset(ones1[:], 1.0)

            # PE pstate warmup: ~3us of junk transposes so the real early
            # matmuls run at full clock instead of the cold p-state.
            if KNOBS["pe_warmup"]:
                psW = psu.tile([P, P], fp16, tag="ps", name="pewarm")
                for _w in range(KNOBS["pe_warmup"]):
                    nc.tensor.transpose(psW[:], ident_pi[:], ident_pi[:])

            # ---- A^T via odd/even-offset fp16 xbar transposes + 1 merge ----
            AT_sb = big.tile([P, NT * N], fp16)
            Vf = A_d.bitcast(fp16)  # [2048, 4096]

            tb_store = {}

            def emit_merge_dmas(k):
                # odd-offset window transposed straight into AT_sb (values on
                # even partitions, zeros on odd); tb staged for the merge.
                # Early k-tiles split into row-halves so the first merge work
                # lands on DVE ~2x sooner.
                dst = AT_sb[:, k * N : (k + 1) * N]
                tb = stream.tile([P, N], fp16, tag="tt",
                                 bufs=KNOBS["tt_bufs"], name=f"tb_{k}")
                if k < KNOBS["quarter_merge_k"]:
                    for q in range(4):
                        lo, hi = q * N // 4, (q + 1) * N // 4
                        nc.sync.dma_start_transpose(
                            dst[:, lo:hi], Vf[lo:hi, 256 * k + 1 : 256 * k + 129])
                        nc.sync.dma_start_transpose(
                            tb[:, lo:hi],
                            Vf[lo:hi, 256 * k + 128 : 256 * k + 256])
                elif k < KNOBS["half_merge_k"]:
                    for lo, hi in ((0, N // 2), (N // 2, N)):
                        nc.sync.dma_start_transpose(
                            dst[:, lo:hi], Vf[lo:hi, 256 * k + 1 : 256 * k + 129])
                        nc.sync.dma_start_transpose(
                            tb[:, lo:hi],
                            Vf[lo:hi, 256 * k + 128 : 256 * k + 256])
                else:
                    nc.sync.dma_start_transpose(
                        dst, Vf[:, 256 * k + 1 : 256 * k + 129])
                    nc.sync.dma_start_transpose(
                        tb[:], Vf[:, 256 * k + 128 : 256 * k + 256])
                tb_store[k] = tb

            def emit_merge_tile(k):
                use_dma = k >= KNOBS["merge_dma_k"]
                use_pool = (not use_dma) and k < KNOBS["merge_pool"]
                dst = AT_sb[:, k * N : (k + 1) * N]
                if k not in tb_store:
                    emit_merge_dmas(k)
                tb = tb_store.pop(k)
                if use_dma:
                    nc.sync.dma_start(
                        AT_sb[:][1:P:2, k * N : (k + 1) * N], tb[:][1:P:2, :])
                elif k < KNOBS["quarter_merge_k"]:
                    eng = nc.gpsimd if use_pool else nc.vector
                    for q in range(4):
                        lo, hi = q * N // 4, (q + 1) * N // 4
                        eng.tensor_tensor(dst[:, lo:hi], dst[:, lo:hi],
                                          tb[:, lo:hi], op.add)
                elif k < KNOBS["half_merge_k"]:
                    eng = nc.gpsimd if use_pool else nc.vector
                    for lo, hi in ((0, N // 2), (N // 2, N)):
                        eng.tensor_tensor(dst[:, lo:hi], dst[:, lo:hi],
                                          tb[:, lo:hi], op.add)
                elif 2 <= k < 2 + KNOBS["merge_split_pool"]:
                    # lo half on DVE, hi half on Pool: sheds merge work to
                    # the Pool engine at half granularity
                    nc.vector.tensor_tensor(dst[:, 0 : N // 2],
                                            dst[:, 0 : N // 2],
                                            tb[:, 0 : N // 2], op.add)
                    nc.gpsimd.tensor_tensor(dst[:, N // 2 : N],
                                            dst[:, N // 2 : N],
                                            tb[:, N // 2 : N], op.add)
                else:
                    eng = nc.gpsimd if use_pool else nc.vector
                    eng.tensor_tensor(dst, dst, tb[:], op.add)

            merged = [0]

            def emit_merges_to(kmax):
                # merges go AFTER each k-group's u/p in the DVE stream: by
                # the time DVE reaches merge k+lead its xbar transposes have
                # long landed, so no head-of-line stall.
                while merged[0] < min(kmax, NT):
                    emit_merge_tile(merged[0])
                    merged[0] += 1

            # WA as fp16 [64, 2H]: wa16[:, 2h:2h+2] = W[h]@[a_self|a_neigh]
            avf = stream.tile([F, 2 * H], f32, tag="avf", bufs=1)
            nc.sync.dma_start(avf.rearrange("f (h two) -> f h two", two=2),
                              wa_d.rearrange("h f two -> f h two"))
            wa16 = const.tile([F, 2 * H], fp16)
            nc.vector.tensor_copy(wa16[:], avf[:])

            # ---- X -> XT16 [65, 2048] (fp16, pi-permuted cols, ones row 64)
            xf = stream.tile([P, NT * F], f32, tag="xf", bufs=1)
            nc.sync.dma_start(
                xf.rearrange("p (t f) -> p t f", f=F),
                X_d.rearrange("(t p) f -> p t f", p=P),
            )
            # first transpose pair(s) directly after the X load: the A xbar
            # stream owns the DMA engines early without delaying the g-chain
            for _k in range(KNOBS["early_pairs"]):
                emit_merge_dmas(_k)
            x16 = stream.tile([P, NT * F], fp16, tag="x16", bufs=1)
            nc.vector.tensor_copy(x16[:], xf[:])
            XT16 = big.tile([F + 1, N], fp16)
            for hx in range(2):
                xTps = psu.tile([F, N // 2], fp16, tag="ps", name=f"xTps_{hx}")
                for j in range(NT // 2):
                    t = hx * (NT // 2) + j
                    nc.tensor.transpose(
                        xTps[:, j * P : (j + 1) * P],
                        x16[:, t * F : (t + 1) * F],
                        ident_pi[:],
                    )
                nc.scalar.copy(XT16[0:F, hx * (N // 2) : (hx + 1) * (N // 2)],
                               xTps[:])
            nc.vector.memset(XT16[F : F + 1, :], 1.0)

            def emit_setup(h):
                Wf = head.tile([F + 1, FH], f32, tag="Wf", bufs=2,
                               name=f"Wf_{h}")
                nc.sync.dma_start(Wf[0:F, :], W_d[h])
                nc.sync.dma_start(Wf[F : F + 1, :], b_d[h : h + 1, :])
                W16 = head.tile([F + 1, FH], fp16, tag="W16", bufs=2,
                                name=f"W16_{h}")
                nc.vector.tensor_copy(W16[:], Wf[:])

                # s_self/s_neigh straight from XT16: s_j[n] = X[n,:]@(W a_j)
                psNg = psu.tile([P, 2 * NT], f32, tag="ps", name=f"psNg_{h}")
                psNg3 = psNg.rearrange("p (k two) -> p k two", two=2)
                for k in range(NT):
                    nc.tensor.matmul(
                        psNg3[:, k, :],
                        XT16[0:F, k * P : (k + 1) * P],
                        wa16[:, 2 * h : 2 * h + 2],
                        start=True, stop=True,
                    )
                # e1 = 0.5*exp(s_neigh), e2 = 0.5*exp(0.2*s_neigh)  (pi rows)
                e1g = head.tile([P, NT], f32, tag="e1g", bufs=4, name=f"e1g_{h}")
                nc.scalar.activation(e1g[:], psNg3[:, :, 1], act.Exp,
                                     scale=1.0, bias=lnhalf[:])
                e2g = head.tile([P, NT], f32, tag="e2g", bufs=4, name=f"e2g_{h}")
                nc.scalar.activation(e2g[:], psNg3[:, :, 1], act.Exp,
                                     scale=0.2, bias=lnhalf[:])
                ssg = head.tile([P, NT], fp16, tag="ssg", bufs=2, name=f"ssg_{h}")
                nc.scalar.copy(ssg[:], psNg3[:, :, 0])
                S = KNOBS["act_split"]
                snb = None
                if S:
                    snb = head.tile([P, NT], f32, tag="snb", bufs=2,
                                    name=f"snb_{h}")
                    nc.scalar.copy(snb[:], psNg3[:, :, 1])

                # g_row natural order: un-permute ssg with ident_pinv
                g_row = head.tile([1, N], fp16, tag="g_row", bufs=2,
                                  name=f"g_row_{h}")
                ss_row = (head.tile([1, S], fp16, tag="ss_row", bufs=2,
                                    name=f"ss_row_{h}") if S else None)
                for c in range(NCH):
                    psRow = psu.tile([1, C], fp16, tag="ps", name=f"psRow_{h}_{c}")
                    for j in range(4):
                        kk = c * 4 + j
                        nc.tensor.transpose(
                            psRow[:, j * P : (j + 1) * P],
                            ssg[:, kk : kk + 1],
                            ident_pinv[:],
                        )
                    nc.scalar.activation(
                        g_row[:, c * C : (c + 1) * C], psRow[:], act.Exp,
                        scale=-0.8,
                    )
                    if S and c * C < S:
                        w = min(C, S - c * C)
                        nc.scalar.copy(ss_row[:, c * C : c * C + w],
                                       psRow[:, 0:w])
                g_bc = head.tile([P, N], fp16, tag="g_bc", bufs=4, name=f"g_bc_{h}")
                if h in KNOBS["gbc_pe_heads"]:
                    # ones-matmul broadcast: g_row streams through the PE and
                    # lands replicated on all partitions (f32 PSUM chunks),
                    # then DVE casts to fp16
                    for c in range(NCH):
                        psB = psu.tile([P, C], f32, tag="ps", name=f"psB_{h}_{c}")
                        nc.tensor.matmul(psB[:], ones1[:],
                                         g_row[:, c * C : (c + 1) * C],
                                         start=True, stop=True)
                        nc.vector.tensor_copy(g_bc[:, c * C : (c + 1) * C],
                                              psB[:])
                elif KNOBS["gbc_dma"] and h not in KNOBS["gbc_pool_heads"]:
                    nc.scalar.dma_start(GROW_d[h : h + 1, :], g_row[:])
                    nc.scalar.dma_start(
                        g_bc[:],
                        GROW_d[h : h + 1, :].partition_broadcast(P).squeeze(1))
                else:
                    nc.gpsimd.partition_broadcast(g_bc[:], g_row[:])

                ss_bc = None
                if S:
                    ss_bc = head.tile([P, S], fp16, tag="ss_bc", bufs=4,
                                      name=f"ss_bc_{h}")
                    nc.scalar.dma_start(SSROW_d[h : h + 1, 0:S], ss_row[:])
                    nc.scalar.dma_start(
                        ss_bc[:],
                        SSROW_d[h : h + 1, 0:S].partition_broadcast(P).squeeze(1))

                G_all = head.tile([P, NT * GW], fp16, tag="G_all", bufs=4,
                                  name=f"G_all_{h}")
                G3 = G_all.rearrange("p (k w) -> p k w", w=GW)
                for halfg in range(2):
                    psG = psu.tile([P, (NT // 2) * FH], f32, tag="ps",
                                   name=f"psG_{h}_{halfg}")
                    for j in range(NT // 2):
                        k = halfg * (NT // 2) + j
                        nc.tensor.matmul(
                            psG[:, j * FH : (j + 1) * FH],
                            XT16[:, k * P : (k + 1) * P],
                            W16[:],
                            start=True, stop=True,
                        )
                    nc.scalar.copy(
                        G3[:, halfg * (NT // 2) : (halfg + 1) * (NT // 2), 0:FH],
                        psG.rearrange("p (k f) -> p k f", f=FH),
                    )
                nc.vector.memset(G3[:, :, FH : FH + 1], 1.0)
                return {"e1g": e1g, "e2g": e2g, "g_bc": g_bc,
                        "G_all": G_all, "agg": None,
                        "snb": snb, "ss_bc": ss_bc}

            def emit_u(h, st, k, use_pool):
                e1g, e2g, g_bc = st["e1g"], st["e2g"], st["g_bc"]
                u_t = stream.tile([P, N], fp16, tag="u", bufs=KNOBS["u_bufs"],
                                  name=f"u_{h}_{k}")
                S = KNOBS["act_split"]
                eng = nc.gpsimd if use_pool else nc.vector
                if S and not use_pool:
                    # columns [0:S] exactly on Act: exp(leaky(ss+sn)); the
                    # per-column softmax factor differs from the DVE half's
                    # convention but cancels in the normalization.
                    tmp = stream.tile([P, S], fp16, tag="uact", bufs=3,
                                      name=f"ua_{h}_{k}")
                    nc.scalar.activation(tmp[:], st["ss_bc"][:], act.Prelu,
                                         bias=st["snb"][:, k : k + 1],
                                         scale=1.0, alpha=alpha02[:])
                    nc.scalar.activation(u_t[:, 0:S], tmp[:], act.Exp)
                    eng.tensor_scalar(
                        u_t[:, S:N], g_bc[:, S:N],
                        e2g[:, k : k + 1], e1g[:, k : k + 1],
                        op.mult, op.max,
                    )
                else:
                    eng.tensor_scalar(
                        u_t[:], g_bc[:],
                        e2g[:, k : k + 1], e1g[:, k : k + 1],
                        op.mult, op.max,
                    )
                return u_t

            def emit_p(h, st, k, u_t, use_pool, tag="p", bufs=None):
                p_t = stream.tile([P, N], fp16, tag=tag,
                                  bufs=bufs or KNOBS["p_bufs"],
                                  name=f"p_{h}_{k}")
                eng = nc.gpsimd if use_pool else nc.vector
                eng.tensor_tensor(
                    p_t[:], u_t[:], AT_sb[:, k * N : (k + 1) * N], op.mult
                )
                return p_t

            def emit_aggs(h, st, k, p_t):
                G_all = st["G_all"]
                if st["agg"] is None:
                    st["agg"] = [
                        psagg.tile([P, g * 65], f32, tag=f"agg{gi}",
                                   name=f"agg{h}_{gi}")
                        for gi, g in enumerate(AGG_GROUPS)
                    ]
                aggs = st["agg"]
                rhs = st["G_all"][:, k * GW : k * GW + 65]
                # PSUM start=True lazily zeroes the whole 2KB bank, so only
                # the FIRST matmul of each bank-group tile may set it; later
                # slices overwrite their pending-zero bytes with start=False.
                for t in range(NT):
                    gi = 0 if t < 7 else (1 if t < 14 else 2)
                    tt_ = t - (0 if t < 7 else (7 if t < 14 else 14))
                    last = AGG_GROUPS[gi] - 1
                    nc.tensor.matmul(
                        aggs[gi][:, tt_ * 65 : tt_ * 65 + 65],
                        p_t[:, t * P : (t + 1) * P],
                        rhs,
                        start=(k == 0 and tt_ == 0),
                        stop=(k == NT - 1 and tt_ == last),
                    )

            def emit_finals(h, st, split_relu=False):
                aggs = st["agg"]
                # den columns (o=64 of each 65-group) -> SBUF, then 1/den
                den = head.tile([P, NT], f32, tag="den", bufs=2,
                                name=f"den_{h}")
                base = 0
                for gi, g in enumerate(AGG_GROUPS):
                    a3 = aggs[gi].rearrange("p (t w) -> p t w", w=65)
                    nc.scalar.copy(den[:, base : base + g], a3[:, :, 64])
                    base += g
                r = head.tile([P, NT], f32, tag="r", bufs=2, name=f"r_{h}")
                nc.vector.reciprocal_approx_fast(r[:], den[:])
                out_sb = outp.tile([P, NT * FH], fp16, tag="outf",
                                   name=f"outf_{h}")
                base = 0
                for gi, g in enumerate(AGG_GROUPS):
                    a3 = aggs[gi].rearrange("p (t w) -> p t w", w=65)
                    for j in range(g):
                        t = base + j
                        if KNOBS["fin_act"] and not (split_relu and t >= KNOBS["split_relu_at"]):
                            nc.scalar.activation(
                                out_sb[:, t * FH : (t + 1) * FH],
                                a3[:, j, 0:FH], act.Relu,
                                scale=r[:, t : t + 1],
                            )
                        else:
                            nc.vector.tensor_scalar(
                                out_sb[:, t * FH : (t + 1) * FH],
                                a3[:, j, 0:FH],
                                r[:, t : t + 1], 0.0, op.mult, op.max,
                            )
                    base += g
                if split_relu:
                    # two half DMAs so the first overlaps the remaining relus
                    ht = NT // 2
                    nc.sync.dma_start(
                        OUT_d[h, 0 : ht * P].rearrange("(t p) f -> p t f", p=P),
                        out_sb.rearrange("p (t f) -> p t f", f=FH)[:, 0:ht, :],
                    )
                    nc.sync.dma_start(
                        OUT_d[h, ht * P : N].rearrange("(t p) f -> p t f", p=P),
                        out_sb.rearrange("p (t f) -> p t f", f=FH)[:, ht:NT, :],
                    )
                else:
                    nc.sync.dma_start(
                        OUT_d[h].rearrange("(t p) f -> p t f", p=P),
                        out_sb.rearrange("p (t f) -> p t f", f=FH),
                    )

            # ---- schedule ------------------------------------------------
            # all four setups run before aggs; heads 0/1 aggregate while A^T
            # streams in, heads 2/3 afterwards.  Pool-assigned u/p ops are
            # emitted with lookahead so the in-order PE agg queue never waits
            # on the slower Pool engine.
            sts = [emit_setup(0), emit_setup(1), None, None]
            seq1 = [(h, k) for k in range(NT) for h in (0, 1)]
            seq2 = [(h, k) for k in range(NT) for h in (2, 3)]

            def pool_set(seq, stride):
                if not stride:
                    return set()
                return {hk for i, hk in enumerate(seq) if i % stride == stride - 1}

            def run_phase(seq, p_pool, u_pool, la, per_k=None, post_k=None,
                          group_done=None):
                pend = {}
                ustore = {}
                emitted = set()
                uahead = KNOBS["u_ahead"]

                def get_u(idx):
                    if idx in ustore:
                        return ustore.pop(idx)
                    h, k = seq[idx]
                    return emit_u(h, sts[h], k, seq[idx] in u_pool)

                def produce_u(idx):
                    # u only needs g_bc: emit ahead so DVE has filler work
                    # while waiting on the A^T merge stream
                    if idx >= len(seq) or idx in ustore or seq[idx] in emitted:
                        return
                    h, k = seq[idx]
                    if sts[h] is None:
                        return
                    ustore[idx] = emit_u(h, sts[h], k, seq[idx] in u_pool)

                def produce(idx):
                    if idx >= len(seq) or seq[idx] in emitted:
                        return
                    h, k = seq[idx]
                    if seq[idx] in p_pool or seq[idx] in u_pool:
                        emitted.add(seq[idx])
                        pend[(h, k)] = emit_p(h, sts[h], k, get_u(idx),
                                              seq[idx] in p_pool)

                lastk = -1
                for i, (h, k) in enumerate(seq):
                    if k != lastk:
                        if per_k is not None:
                            per_k(k)
                        if post_k is not None and lastk >= 0:
                            post_k(lastk)
                        lastk = k
                    for j in range(i, min(i + uahead + 1, len(seq))):
                        produce_u(j)
                    for j in range(i, min(i + la + 1, len(seq))):
                        produce(j)
                    if (h, k) in pend:
                        emit_aggs(h, sts[h], k, pend.pop((h, k)))
                    else:
                        emit_aggs(h, sts[h], k,
                                  emit_p(h, sts[h], k, get_u(i), False))
                    if group_done is not None and (i + 1 == len(seq)
                                                   or seq[i + 1][1] != k):
                        group_done(k)
                if post_k is not None and lastk >= 0:
                    post_k(lastk)

            lead = KNOBS["lead"]

            def per_k1(k):
                if k == 0:
                    emit_merges_to(KNOBS["prologue_merges"])
                if k == KNOBS["setup2_k"]:
                    sts[2] = emit_setup(2)
                if k == KNOBS["setup3_k"]:
                    sts[3] = emit_setup(3)

            def post_k1(k):
                emit_merges_to(k + lead + 2)

            pool1 = pool_set(seq1, KNOBS["p_pool_1"])
            pool2 = pool_set(seq2, KNOBS["p_pool_2"])
            upool1 = pool_set(list(reversed(seq1)), KNOBS["u_pool_1"])
            upool2 = pool_set(list(reversed(seq2)), KNOBS["u_pool_2"])

            # prefill: during phase-1 tail, pre-build head-2 u/p for early k
            # (aggs deferred until its PSUM frees after finals(0)/finals(1))
            prefill = KNOBS["prefill"]
            p2_store = {}

            def group_done1(k):
                j = k - (NT - prefill)
                if 0 <= j < prefill and sts[2] is not None:
                    u2 = emit_u(2, sts[2], j, False)
                    p2_store[j] = emit_p(2, sts[2], j, u2, False,
                                         tag="p2", bufs=max(prefill, 1))

            run_phase(seq1, pool1, upool1, KNOBS["pool_la"], per_k=per_k1,
                      post_k=post_k1, group_done=group_done1)
            emit_finals(0, sts[0])
            emit_finals(1, sts[1])
            seq2a = [(2, k) for k in range(NT) if k not in p2_store]
            seq2b = [(3, k) for k in range(NT)]
            for j in sorted(p2_store):
                emit_aggs(2, sts[2], j, p2_store.pop(j))
            if KNOBS["pool2_split"]:
                # independent pool strides per sub-phase (head 2 vs head 3)
                pool2a = pool_set(seq2a, KNOBS["p_pool_2"])
                pool2b = pool_set(seq2b, KNOBS["p_pool_2b"])
                upool2a = pool_set(list(reversed(seq2a)), KNOBS["u_pool_2"])
                upool2b = pool_set(list(reversed(seq2b)), KNOBS["u_pool_2b"])
            else:
                pool2a = {hk for hk in pool2 if hk[0] == 2 and hk[1] >= prefill}
                pool2b = {hk for hk in pool2 if hk[0] == 3}
                upool2a = {hk for hk in upool2 if hk[0] == 2 and hk[1] >= prefill}
                upool2b = {hk for hk in upool2 if hk[0] == 3}
            run_phase(seq2a, pool2a, upool2a, KNOBS["pool_la"])
            emit_finals(2, sts[2])
            run_phase(seq2b, pool2b, upool2b, KNOBS["pool_la"])
            emit_finals(3, sts[3], split_relu=True)

    nc.compile()
    return nc


def _get_nc():
    if "nc" not in _CACHE:
        _CACHE["nc"] = _build()
    return _CACHE["nc"]


def make_in_maps(inputs):
    X = np.ascontiguousarray(inputs["X"], dtype=np.float32)
    A = np.ascontiguousarray(inputs["A"], dtype=np.float32)
    W = np.ascontiguousarray(inputs["W"], dtype=np.float32)
    b = np.ascontiguousarray(inputs["b"], dtype=np.float32)
    a_self = np.ascontiguousarray(inputs["a_self"], dtype=np.float32)
    a_neigh = np.ascontiguousarray(inputs["a_neigh"], dtype=np.float32)
    # tiny host precompute: WA[h] = [W[h]@a_self[h] | W[h]@a_neigh[h]]
    WA = np.ascontiguousarray(
        np.stack([np.einsum("hfo,ho->hf", W, a_self),
                  np.einsum("hfo,ho->hf", W, a_neigh)], axis=2),
        dtype=np.float32)
    return [
        {
            "A": np.ascontiguousarray(A[i]),
            "X": np.ascontiguousarray(X[i]),
            "W": W,
            "b": b,
            "WA": WA,
        }
        for i in range(B)
    ]


def run(inputs, trace=False):
    from concourse import bass_utils

    nc = _get_nc()
    in_maps = make_in_maps(inputs)
    res = bass_utils.run_bass_kernel_spmd(
        nc, in_maps, core_ids=list(range(B)), trace=trace
    )
    out = np.empty((B, N, H * FH), dtype=np.float32)
    for i in range(B):
        o = np.asarray(res.results[i]["OUT"], dtype=np.float32)  # [H, N, FH]
        out[i] = o.transpose(1, 0, 2).reshape(N, H * FH)
    return out, res


def kernel(**inputs):
    out, _ = run(inputs, trace=False)
    return out


# revision 64
# speedup vs baseline: 1.0103x; 1.0103x over previous
"""Batch graph attention (GAT-style) Trainium2 kernel.

Problem: B=8, N=2048, F=64, FH=64, H=4.
  feats = X @ W[h]                         [B,H,N,FH]
  scores[n,m] = leaky_relu(s_self[n] + s_neigh[m], 0.2)
  P = softmax(scores + (1-A)*NEG_BIG, axis=m)
  out = relu(concat_h(P @ feats + b))

Sharding: batch b -> core b (8 cores, data parallel).

Per-core algorithm (neighbor index m on SBUF partitions):

  exp(leaky(x)) == max(e^x, e^{0.2x})  (slope<1); dropping the per-column
  factor e^{s_self[n]} (softmax columns are scale invariant) leaves

      Phat[m,n] = A^T[m,n] * max(e1[m], e2[m] * g[n])

  with e1=0.5*exp(s_neigh), e2=0.5*exp(0.2*s_neigh), g=exp(-0.8*s_self).
  Per (h,k-tile): u = (g_bc * e2) max e1 (DVE ts, 4x) and p = u * A^T
  (DVE tt, 2x).  Aggregation uses the TRANSPOSED matmul orientation:
  p-chunk [128m x 128n] is the PE stationary, G = [feats+b | 1] the
  65-col moving operand, accumulating agg[n, o] (+den at o=64) in PSUM
  per (head, ntile).  That makes den a per-partition column, so finals
  are: den cols -> SBUF (Act), one DVE reciprocal per head, and a
  per-ntile Act Relu(scale=1/den) straight out of PSUM.  Output leaves
  in natural [H, N, FH] orientation (host concatenates heads).

  A^T comes from fp32 A's fp16 bit-pair structure: fp16 view of fp32 1.0
  is [0x0000 | 0x3F80] = [0 | 1.875].  An xbar DMA transpose of 128 fp16
  columns starting at an ODD offset lands the 1.875*A values on EVEN
  output partitions (zeros on odd); the aligned window starting 128 later
  lands its values on ODD partitions.  A merge (DVE add, Pool add, or a
  partition-strided SBUF-to-SBUF DMA) produces dense 1.875*A^T with rows
  in the fixed interleave pi(p) = p/2 (p even) | 64+(p-1)/2 (p odd).
  The 1.875 cancels in the softmax; the pi permutation is absorbed by
  building XT16's columns pi-permuted, so G rows / e-vectors line up.
  g (an n-indexed row) is un-permuted during its PE transpose with the
  inverse identity.
"""

import numpy as np

B, N, F, FH, H = 8, 2048, 64, 64, 4
P = 128           # SBUF partitions
NT = N // P       # 16 m-tiles / n-tiles
C = 512           # chunk used for feats matmuls
NCH = N // C      # 4 chunks
GW = 66           # G row stride (64 feats + 1 ones + 1 pad)
LN_HALF = -0.6931471805599453

_CACHE = {}

# tuning knobs (read at build time)
KNOBS = {
    "tt_bufs": 3,         # xbar staging tile buffers
    "u_bufs": 14,
    "u_ahead": 10,         # emit u-ops this many seq steps ahead (needs u_bufs)
    "p_bufs": 5,
    "outp_bufs": 1,
    "lead": 2,            # merge lead (in k) ahead of consumption
    "pool_la": 4,         # lookahead (in seq steps) for pool-assigned ops
    "merge_dma_k": 16,    # merges with k >= this go via DMA
    "merge_pool": 0,      # first k merges on Pool (rest DVE)
    "prologue_merges": 5, # merges emitted before the first k-group
    "p_pool_1": 3,        # phase-1: every n-th (h,k) p-op on Pool
    "p_pool_2": 5,        # phase-2: every n-th (h,k) p-op on Pool
    "u_pool_1": 10,        # phase-1: every n-th (h,k) u-op on Pool
    "u_pool_2": 7,        # phase-2: every n-th (h,k) u-op on Pool
    "pool2_split": 0,     # independent per-sub-phase pool strides below
    "p_pool_2b": 3,
    "u_pool_2b": 4,
    "gbc_dma": True,      # g broadcast via DRAM bounce (else Pool)
    "gbc_pool_heads": (0, 1, 2),  # heads whose g broadcast goes on Pool
    "gbc_pe_heads": (),   # heads whose g broadcast goes via PE ones-matmul
    "fin_act": True,      # final relu/scale on Act (else DVE)
    "split_relu_at": 6,   # last head: relus with t >= this go on DVE
    "setup2_k": 14,        # phase-1 k at which head-2 setup is emitted
    "setup3_k": 15,       # phase-1 k at which head-3 setup is emitted
    "pe_warmup": 0,       # junk PE transposes for p-state ramp (0=off)
    "early_pairs": 0,     # merge tiles whose transposes precede X/W loads
    "half_merge_k": 2,    # first k merges done in column halves
    "quarter_merge_k": 0, # first k merges done in column quarters
    "merge_split_pool": 0,  # merges k in [2,2+n): lo half DVE, hi half Pool
    "prefill": 0,         # head-2 k-groups pre-built during phase-1 tail
    "act_split": 0,       # leading columns of u computed on Act (Prelu+Exp)
}

# agg PSUM bank grouping: ntiles per PSUM tile (65 f32 cols each; 7*65*4B
# = 1820B fits one 2KB bank, so slices never cross a bank boundary)
AGG_GROUPS = (7, 7, 2)


def _build():
    import concourse.bacc as bacc
    import concourse.tile as tile
    import concourse.mybir as mybir
    from concourse.mybir import AluOpType as op, ActivationFunctionType as act

    f32 = mybir.dt.float32
    fp16 = mybir.dt.float16
    i32 = mybir.dt.int32

    nc = bacc.Bacc(
        "TRN2",
        target_bir_lowering=False,
        debug=False,
        enable_asserts=False,
        num_devices=8,
    )

    A_d = nc.dram_tensor("A", [N, N], f32, kind="ExternalInput").ap()
    X_d = nc.dram_tensor("X", [N, F], f32, kind="ExternalInput").ap()
    W_d = nc.dram_tensor("W", [H, F, FH], f32, kind="ExternalInput").ap()
    b_d = nc.dram_tensor("b", [H, FH], f32, kind="ExternalInput").ap()
    # WA[h] = [W[h] @ a_self[h] | W[h] @ a_neigh[h]]  (tiny, host-precomputed)
    wa_d = nc.dram_tensor("WA", [H, F, 2], f32, kind="ExternalInput").ap()
    OUT_d = nc.dram_tensor("OUT", [H, N, FH], fp16, kind="ExternalOutput").ap()
    GROW_d = nc.dram_tensor("GROW", [H, N], fp16, kind="Internal").ap()
    SSROW_d = nc.dram_tensor("SSROW", [H, N], fp16, kind="Internal").ap()

    with tile.TileContext(nc) as tc:
        with (
            tc.tile_pool(name="const", bufs=1) as const,
            tc.tile_pool(name="big", bufs=1) as big,
            tc.tile_pool(name="stream", bufs=3) as stream,
            tc.tile_pool(name="head", bufs=2) as head,
            tc.tile_pool(name="outp", bufs=KNOBS["outp_bufs"]) as outp,
            tc.tile_pool(name="psu", bufs=2, space="PSUM") as psu,
            tc.tile_pool(name="psagg", bufs=2, space="PSUM") as psagg,
        ):
            # ---- constants --------------------------------------------
            iota_i = stream.tile([P, P], i32, tag="cst", bufs=2)
            nc.gpsimd.iota(iota_i[:], pattern=[[1, P]], base=0, channel_multiplier=0)
            pidx_i = stream.tile([P, 1], i32, tag="cst1", bufs=1)
            nc.gpsimd.iota(pidx_i[:], pattern=[[0, 1]], base=0, channel_multiplier=1)
            pidx_f = const.tile([P, 1], f32)
            nc.vector.tensor_copy(pidx_f[:], pidx_i[:])
            iota_f = stream.tile([P, P], f32, tag="cstf", bufs=2)
            nc.vector.tensor_copy(iota_f[:], iota_i[:])
            # pi sequence [0,64,1,65,...]: ident_pi[p,c]=1 iff p==pi(c)
            iopi_i = stream.tile([P, P], i32, tag="cst", bufs=2)
            nc.gpsimd.iota(iopi_i[:], pattern=[[1, 64], [64, 2]], base=0,
                           channel_multiplier=0)
            iopi_f = stream.tile([P, P], f32, tag="cstf", bufs=2)
            nc.vector.tensor_copy(iopi_f[:], iopi_i[:])
            ident_pi = const.tile([P, P], fp16)
            nc.vector.tensor_scalar(ident_pi[:], iopi_f[:], pidx_f[:], None,
                                    op.is_equal)
            # pinv sequence [0,2,..126,1,3,..127]: ident_pinv[p,c]=1 iff p==pinv(c)
            iopv_i = stream.tile([P, P], i32, tag="cst", bufs=2)
            nc.gpsimd.iota(iopv_i[:], pattern=[[1, 2], [2, 64]], base=0,
                           channel_multiplier=0)
            iopv_f = stream.tile([P, P], f32, tag="cstf", bufs=2)
            nc.vector.tensor_copy(iopv_f[:], iopv_i[:])
            ident_pinv = const.tile([P, P], fp16)
            nc.vector.tensor_scalar(ident_pinv[:], iopv_f[:], pidx_f[:], None,
                                    op.is_equal)

            lnhalf = const.tile([P, 1], f32)
            nc.vector.memset(lnhalf[:], LN_HALF)
            alpha02 = const.tile([P, 1], f32)
            nc.vector.memset(alpha02[:], 0.2)
            ones1 = const.tile([1, P], fp16)
            nc.vector.memset(ones1[:], 1.0)

            # PE pstate warmup: ~3us of junk transposes so the real early
            # matmuls run at full clock instead of the cold p-state.
            if KNOBS["pe_warmup"]:
                psW = psu.tile([P, P], fp16, tag="ps", name="pewarm")
                for _w in range(KNOBS["pe_warmup"]):
                    nc.tensor.transpose(psW[:], ident_pi[:], ident_pi[:])

            # ---- A^T via odd/even-offset fp16 xbar transposes + 1 merge ----
            AT_sb = big.tile([P, NT * N], fp16)
            Vf = A_d.bitcast(fp16)  # [2048, 4096]

            tb_store = {}

            def emit_merge_dmas(k):
                # odd-offset window transposed straight into AT_sb (values on
                # even partitions, zeros on odd); tb staged for the merge.
                # Early k-tiles split into row-halves so the first merge work
                # lands on DVE ~2x sooner.
                dst = AT_sb[:, k * N : (k + 1) * N]
                tb = stream.tile([P, N], fp16, tag="tt",
                                 bufs=KNOBS["tt_bufs"], name=f"tb_{k}")
                if k < KNOBS["quarter_merge_k"]:
                    for q in range(4):
                        lo, hi = q * N // 4, (q + 1) * N // 4
                        nc.sync.dma_start_transpose(
                            dst[:, lo:hi], Vf[lo:hi, 256 * k + 1 : 256 * k + 129])
                        nc.sync.dma_start_transpose(
                            tb[:, lo:hi],
                            Vf[lo:hi, 256 * k + 128 : 256 * k + 256])
                elif k < KNOBS["half_merge_k"]:
                    for lo, hi in ((0, N // 2), (N // 2, N)):
                        nc.sync.dma_start_transpose(
                            dst[:, lo:hi], Vf[lo:hi, 256 * k + 1 : 256 * k + 129])
                        nc.sync.dma_start_transpose(
                            tb[:, lo:hi],
                            Vf[lo:hi, 256 * k + 128 : 256 * k + 256])
                else:
                    nc.sync.dma_start_transpose(
                        dst, Vf[:, 256 * k + 1 : 256 * k + 129])
                    nc.sync.dma_start_transpose(
                        tb[:], Vf[:, 256 * k + 128 : 256 * k + 256])
                tb_store[k] = tb

            def emit_merge_tile(k):
                use_dma = k >= KNOBS["merge_dma_k"]
                use_pool = (not use_dma) and k < KNOBS["merge_pool"]
                dst = AT_sb[:, k * N : (k + 1) * N]
                if k not in tb_store:
                    emit_merge_dmas(k)
                tb = tb_store.pop(k)
                if use_dma:
                    nc.sync.dma_start(
                        AT_sb[:][1:P:2, k * N : (k + 1) * N], tb[:][1:P:2, :])
                elif k < KNOBS["quarter_merge_k"]:
                    eng = nc.gpsimd if use_pool else nc.vector
                    for q in range(4):
                        lo, hi = q * N // 4, (q + 1) * N // 4
                        eng.tensor_tensor(dst[:, lo:hi], dst[:, lo:hi],
                                          tb[:, lo:hi], op.add)
                elif k < KNOBS["half_merge_k"]:
                    eng = nc.gpsimd if use_pool else nc.vector
                    for lo, hi in ((0, N // 2), (N // 2, N)):
                        eng.tensor_tensor(dst[:, lo:hi], dst[:, lo:hi],
                                          tb[:, lo:hi], op.add)
                elif 2 <= k < 2 + KNOBS["merge_split_pool"]:
                    # lo half on DVE, hi half on Pool: sheds merge work to
                    # the Pool engine at half granularity
                    nc.vector.tensor_tensor(dst[:, 0 : N // 2],
                                            dst[:, 0 : N // 2],
                                            tb[:, 0 : N // 2], op.add)
                    nc.gpsimd.tensor_tensor(dst[:, N // 2 : N],
                                            dst[:, N // 2 : N],
                                            tb[:, N // 2 : N], op.add)
                else:
                    eng = nc.gpsimd if use_pool else nc.vector
                    eng.tensor_tensor(dst, dst, tb[:], op.add)

            merged = [0]

            def emit_merges_to(kmax):
                # merges go AFTER each k-group's u/p in the DVE stream: by
                # the time DVE reaches merge k+lead its xbar transposes have
                # long landed, so no head-of-line stall.
                while merged[0] < min(kmax, NT):
                    emit_merge_tile(merged[0])
                    merged[0] += 1

            # WA as fp16 [64, 2H]: wa16[:, 2h:2h+2] = W[h]@[a_self|a_neigh]
            avf = stream.tile([F, 2 * H], f32, tag="avf", bufs=1)
            nc.sync.dma_start(avf.rearrange("f (h two) -> f h two", two=2),
                              wa_d.rearrange("h f two -> f h two"))
            wa16 = const.tile([F, 2 * H], fp16)
            nc.vector.tensor_copy(wa16[:], avf[:])

            # ---- X -> XT16 [65, 2048] (fp16, pi-permuted cols, ones row 64)
            xf = stream.tile([P, NT * F], f32, tag="xf", bufs=1)
            nc.sync.dma_start(
                xf.rearrange("p (t f) -> p t f", f=F),
                X_d.rearrange("(t p) f -> p t f", p=P),
            )
            # first transpose pair(s) directly after the X load: the A xbar
            # stream owns the DMA engines early without delaying the g-chain
            for _k in range(KNOBS["early_pairs"]):
                emit_merge_dmas(_k)
            x16 = stream.tile([P, NT * F], fp16, tag="x16", bufs=1)
            nc.vector.tensor_copy(x16[:], xf[:])
            XT16 = big.tile([F + 1, N], fp16)
            for hx in range(2):
                xTps = psu.tile([F, N // 2], fp16, tag="ps", name=f"xTps_{hx}")
                for j in range(NT // 2):
                    t = hx * (NT // 2) + j
                    nc.tensor.transpose(
                        xTps[:, j * P : (j + 1) * P],
                        x16[:, t * F : (t + 1) * F],
                        ident_pi[:],
                    )
                nc.scalar.copy(XT16[0:F, hx * (N // 2) : (hx + 1) * (N // 2)],
                               xTps[:])
            nc.vector.memset(XT16[F : F + 1, :], 1.0)

            def emit_setup(h):
                Wf = head.tile([F + 1, FH], f32, tag="Wf", bufs=2,
                               name=f"Wf_{h}")
                nc.sync.dma_start(Wf[0:F, :], W_d[h])
                nc.sync.dma_start(Wf[F : F + 1, :], b_d[h : h + 1, :])
                W16 = head.tile([F + 1, FH], fp16, tag="W16", bufs=2,
                                name=f"W16_{h}")
                nc.vector.tensor_copy(W16[:], Wf[:])

                # s_self/s_neigh straight from XT16: s_j[n] = X[n,:]@(W a_j)
                psNg = psu.tile([P, 2 * NT], f32, tag="ps", name=f"psNg_{h}")
                psNg3 = psNg.rearrange("p (k two) -> p k two", two=2)
                for k in range(NT):
                    nc.tensor.matmul(
                        psNg3[:, k, :],
                        XT16[0:F, k * P : (k + 1) * P],
                        wa16[:, 2 * h : 2 * h + 2],
                        start=True, stop=True,
                    )
                # e1 = 0.5*exp(s_neigh), e2 = 0.5*exp(0.2*s_neigh)  (pi rows)
                e1g = head.tile([P, NT], f32, tag="e1g", bufs=4, name=f"e1g_{h}")
                nc.scalar.activation(e1g[:], psNg3[:, :, 1], act.Exp,
                                     scale=1.0, bias=lnhalf[:])
                e2g = head.tile([P, NT], f32, tag="e2g", bufs=4, name=f"e2g_{h}")
                nc.scalar.activation(e2g[:], psNg3[:, :, 1], act.Exp,
                                     scale=0.2, bias=lnhalf[:])
                ssg = head.tile([P, NT], fp16, tag="ssg", bufs=2, name=f"ssg_{h}")
                nc.scalar.copy(ssg[:], psNg3[:, :, 0])
                S = KNOBS["act_split"]
                snb = None
                if S:
                    snb = head.tile([P, NT], f32, tag="snb", bufs=2,
                                    name=f"snb_{h}")
                    nc.scalar.copy(snb[:], psNg3[:, :, 1])

                # g_row natural order: un-permute ssg with ident_pinv
                g_row = head.tile([1, N], fp16, tag="g_row", bufs=2,
                                  name=f"g_row_{h}")
                ss_row = (head.tile([1, S], fp16, tag="ss_row", bufs=2,
                                    name=f"ss_row_{h}") if S else None)
                for c in range(NCH):
                    psRow = psu.tile([1, C], fp16, tag="ps", name=f"psRow_{h}_{c}")
                    for j in range(4):
                        kk = c * 4 + j
                        nc.tensor.transpose(
                            psRow[:, j * P : (j + 1) * P],
                            ssg[:, kk : kk + 1],
                            ident_pinv[:],
                        )
                    nc.scalar.activation(
                        g_row[:, c * C : (c + 1) * C], psRow[:], act.Exp,
                        scale=-0.8,
                    )
                    if S and c * C < S:
                        w = min(C, S - c * C)
                        nc.scalar.copy(ss_row[:, c * C : c * C + w],
                                       psRow[:, 0:w])
                g_bc = head.tile([P, N], fp16, tag="g_bc", bufs=4, name=f"g_bc_{h}")
                if h in KNOBS["gbc_pe_heads"]:
                    # ones-matmul broadcast: g_row streams through the PE and
                    # lands replicated on all partitions (f32 PSUM chunks),
                    # then DVE casts to fp16
                    for c in range(NCH):
                        psB = psu.tile([P, C], f32, tag="ps", name=f"psB_{h}_{c}")
                        nc.tensor.matmul(psB[:], ones1[:],
                                         g_row[:, c * C : (c + 1) * C],
                                         start=True, stop=True)
                        nc.vector.tensor_copy(g_bc[:, c * C : (c + 1) * C],
                                              psB[:])
                elif KNOBS["gbc_dma"] and h not in KNOBS["gbc_pool_heads"]:
                    nc.scalar.dma_start(GROW_d[h : h + 1, :], g_row[:])
                    nc.scalar.dma_start(
                        g_bc[:],
                        GROW_d[h : h + 1, :].partition_broadcast(P).squeeze(1))
                else:
                    nc.gpsimd.partition_broadcast(g_bc[:], g_row[:])

                ss_bc = None
                if S:
                    ss_bc = head.tile([P, S], fp16, tag="ss_bc", bufs=4,
                                      name=f"ss_bc_{h}")
                    nc.scalar.dma_start(SSROW_d[h : h + 1, 0:S], ss_row[:])
                    nc.scalar.dma_start(
                        ss_bc[:],
                        SSROW_d[h : h + 1, 0:S].partition_broadcast(P).squeeze(1))

                G_all = head.tile([P, NT * GW], fp16, tag="G_all", bufs=4,
                                  name=f"G_all_{h}")
                G3 = G_all.rearrange("p (k w) -> p k w", w=GW)
                for halfg in range(2):
                    psG = psu.tile([P, (NT // 2) * FH], f32, tag="ps",
                                   name=f"psG_{h}_{halfg}")
                    for j in range(NT // 2):
                        k = halfg * (NT // 2) + j
                        nc.tensor.matmul(
                            psG[:, j * FH : (j + 1) * FH],
                            XT16[:, k * P : (k + 1) * P],
                            W16[:],
                            start=True, stop=True,
                        )
                    nc.scalar.copy(
                        G3[:, halfg * (NT // 2) : (halfg + 1) * (NT // 2), 0:FH],
                        psG.rearrange("p (k f) -> p k f", f=FH),
                    )
                nc.vector.memset(G3[:, :, FH : FH + 1], 1.0)
                return {"e1g": e1g, "e2g": e2g, "g_bc": g_bc,
                        "G_all": G_all, "agg": None,
                        "snb": snb, "ss_bc": ss_bc}

            def emit_u(h, st, k, use_pool):
                e1g, e2g, g_bc = st["e1g"], st["e2g"], st["g_bc"]
                u_t = stream.tile([P, N], fp16, tag="u", bufs=KNOBS["u_bufs"],
                                  name=f"u_{h}_{k}")
                S = KNOBS["act_split"]
                eng = nc.gpsimd if use_pool else nc.vector
                if S and not use_pool:
                    # columns [0:S] exactly on Act: exp(leaky(ss+sn)); the
                    # per-column softmax factor differs from the DVE half's
                    # convention but cancels in the normalization.
                    tmp = stream.tile([P, S], fp16, tag="uact", bufs=3,
                                      name=f"ua_{h}_{k}")
                    nc.scalar.activation(tmp[:], st["ss_bc"][:], act.Prelu,
                                         bias=st["snb"][:, k : k + 1],
                                         scale=1.0, alpha=alpha02[:])
                    nc.scalar.activation(u_t[:, 0:S], tmp[:], act.Exp)
                    eng.tensor_scalar(
                        u_t[:, S:N], g_bc[:, S:N],
                        e2g[:, k : k + 1], e1g[:, k : k + 1],
                        op.mult, op.max,
                    )
                else:
                    eng.tensor_scalar(
                        u_t[:], g_bc[:],
                        e2g[:, k : k + 1], e1g[:, k : k + 1],
                        op.mult, op.max,
                    )
                return u_t

            def emit_p(h, st, k, u_t, use_pool, tag="p", bufs=None):
                p_t = stream.tile([P, N], fp16, tag=tag,
                                  bufs=bufs or KNOBS["p_bufs"],
                                  name=f"p_{h}_{k}")
                eng = nc.gpsimd if use_pool else nc.vector
                eng.tensor_tensor(
                    p_t[:], u_t[:], AT_sb[:, k * N : (k + 1) * N], op.mult
                )
                return p_t

            def emit_aggs(h, st, k, p_t):
                G_all = st["G_all"]
                if st["agg"] is None:
                    st["agg"] = [
                        psagg.tile([P, g * 65], f32, tag=f"agg{gi}",
                                   name=f"agg{h}_{gi}")
                        for gi, g in enumerate(AGG_GROUPS)
                    ]
                aggs = st["agg"]
                rhs = st["G_all"][:, k * GW : k * GW + 65]
                # PSUM start=True lazily zeroes the whole 2KB bank, so only
                # the FIRST matmul of each bank-group tile may set it; later
                # slices overwrite their pending-zero bytes with start=False.
                for t in range(NT):
                    gi = 0 if t < 7 else (1 if t < 14 else 2)
                    tt_ = t - (0 if t < 7 else (7 if t < 14 else 14))
                    last = AGG_GROUPS[gi] - 1
                    nc.tensor.matmul(
                        aggs[gi][:, tt_ * 65 : tt_ * 65 + 65],
                        p_t[:, t * P : (t + 1) * P],
                        rhs,
                        start=(k == 0 and tt_ == 0),
                        stop=(k == NT - 1 and tt_ == last),
                    )

            def emit_finals(h, st, split_relu=False):
                aggs = st["agg"]
                # den columns (o=64 of each 65-group) -> SBUF, then 1/den
                den = head.tile([P, NT], f32, tag="den", bufs=2,
                                name=f"den_{h}")
                base = 0
                for gi, g in enumerate(AGG_GROUPS):
                    a3 = aggs[gi].rearrange("p (t w) -> p t w", w=65)
                    nc.scalar.copy(den[:, base : base + g], a3[:, :, 64])
                    base += g
                r = head.tile([P, NT], f32, tag="r", bufs=2, name=f"r_{h}")
                nc.vector.reciprocal_approx_fast(r[:], den[:])
                out_sb = outp.tile([P, NT * FH], fp16, tag="outf",
                                   name=f"outf_{h}")
                base = 0
                for gi, g in enumerate(AGG_GROUPS):
                    a3 = aggs[gi].rearrange("p (t w) -> p t w", w=65)
                    for j in range(g):
                        t = base + j
                        if KNOBS["fin_act"] and not (split_relu and t >= KNOBS["split_relu_at"]):
                            nc.scalar.activation(
                                out_sb[:, t * FH : (t + 1) * FH],
                                a3[:, j, 0:FH], act.Relu,
                                scale=r[:, t : t + 1],
                            )
                        else:
                            nc.vector.tensor_scalar(
                                out_sb[:, t * FH : (t + 1) * FH],
                                a3[:, j, 0:FH],
                                r[:, t : t + 1], 0.0, op.mult, op.max,
                            )
                    base += g
                if split_relu:
                    # two half DMAs so the first overlaps the remaining relus
                    ht = NT // 2
                    nc.sync.dma_start(
                        OUT_d[h, 0 : ht * P].rearrange("(t p) f -> p t f", p=P),
                        out_sb.rearrange("p (t f) -> p t f", f=FH)[:, 0:ht, :],
                    )
                    nc.sync.dma_start(
                        OUT_d[h, ht * P : N].rearrange("(t p) f -> p t f", p=P),
                        out_sb.rearrange("p (t f) -> p t f", f=FH)[:, ht:NT, :],
                    )
                else:
                    nc.sync.dma_start(
                        OUT_d[h].rearrange("(t p) f -> p t f", p=P),
                        out_sb.rearrange("p (t f) -> p t f", f=FH),
                    )

            # ---- schedule ------------------------------------------------
            # all four setups run before aggs; heads 0/1 aggregate while A^T
            # streams in, heads 2/3 afterwards.  Pool-assigned u/p ops are
            # emitted with lookahead so the in-order PE agg queue never waits
            # on the slower Pool engine.
            sts = [emit_setup(0), emit_setup(1), None, None]
            seq1 = [(h, k) for k in range(NT) for h in (0, 1)]
            seq2 = [(h, k) for k in range(NT) for h in (2, 3)]

            def pool_set(seq, stride):
                if not stride:
                    return set()
                return {hk for i, hk in enumerate(seq) if i % stride == stride - 1}

            def run_phase(seq, p_pool, u_pool, la, per_k=None, post_k=None,
                          group_done=None):
                pend = {}
                ustore = {}
                emitted = set()
                uahead = KNOBS["u_ahead"]

                def get_u(idx):
                    if idx in ustore:
                        return ustore.pop(idx)
                    h, k = seq[idx]
                    return emit_u(h, sts[h], k, seq[idx] in u_pool)

                def produce_u(idx):
                    # u only needs g_bc: emit ahead so DVE has filler work
                    # while waiting on the A^T merge stream
                    if idx >= len(seq) or idx in ustore or seq[idx] in emitted:
                        return
                    h, k = seq[idx]
                    if sts[h] is None:
                        return
                    ustore[idx] = emit_u(h, sts[h], k, seq[idx] in u_pool)

                def produce(idx):
                    if idx >= len(seq) or seq[idx] in emitted:
                        return
                    h, k = seq[idx]
                    if seq[idx] in p_pool or seq[idx] in u_pool:
                        emitted.add(seq[idx])
                        pend[(h, k)] = emit_p(h, sts[h], k, get_u(idx),
                                              seq[idx] in p_pool)

                lastk = -1
                for i, (h, k) in enumerate(seq):
                    if k != lastk:
                        if per_k is not None:
                            per_k(k)
                        if post_k is not None and lastk >= 0:
                            post_k(lastk)
                        lastk = k
                    for j in range(i, min(i + uahead + 1, len(seq))):
                        produce_u(j)
                    for j in range(i, min(i + la + 1, len(seq))):
                        produce(j)
                    if (h, k) in pend:
                        emit_aggs(h, sts[h], k, pend.pop((h, k)))
                    else:
                        emit_aggs(h, sts[h], k,
                                  emit_p(h, sts[h], k, get_u(i), False))
                    if group_done is not None and (i + 1 == len(seq)
                                                   or seq[i + 1][1] != k):
                        group_done(k)
                if post_k is not None and lastk >= 0:
                    post_k(lastk)

            lead = KNOBS["lead"]

            def per_k1(k):
                if k == 0:
                    emit_merges_to(KNOBS["prologue_merges"])
                if k == KNOBS["setup2_k"]:
                    sts[2] = emit_setup(2)
                if k == KNOBS["setup3_k"]:
                    sts[3] = emit_setup(3)

            def post_k1(k):
                emit_merges_to(k + lead + 2)

            pool1 = pool_set(seq1, KNOBS["p_pool_1"])
            pool2 = pool_set(seq2, KNOBS["p_pool_2"])
            upool1 = pool_set(list(reversed(seq1)), KNOBS["u_pool_1"])
            upool2 = pool_set(list(reversed(seq2)), KNOBS["u_pool_2"])

            # prefill: during phase-1 tail, pre-build head-2 u/p for early k
            # (aggs deferred until its PSUM frees after finals(0)/finals(1))
            prefill = KNOBS["prefill"]
            p2_store = {}

            def group_done1(k):
                j = k - (NT - prefill)
                if 0 <= j < prefill and sts[2] is not None:
                    u2 = emit_u(2, sts[2], j, False)
                    p2_store[j] = emit_p(2, sts[2], j, u2, False,
                                         tag="p2", bufs=max(prefill, 1))

            run_phase(seq1, pool1, upool1, KNOBS["pool_la"], per_k=per_k1,
                      post_k=post_k1, group_done=group_done1)
            emit_finals(0, sts[0])
            emit_finals(1, sts[1])
            seq2a = [(2, k) for k in range(NT) if k not in p2_store]
            seq2b = [(3, k) for k in range(NT)]
            for j in sorted(p2_store):
                emit_aggs(2, sts[2], j, p2_store.pop(j))
            if KNOBS["pool2_split"]:
                # independent pool strides per sub-phase (head 2 vs head 3)
                pool2a = pool_set(seq2a, KNOBS["p_pool_2"])
                pool2b = pool_set(seq2b, KNOBS["p_pool_2b"])
                upool2a = pool_set(list(reversed(seq2a)), KNOBS["u_pool_2"])
                upool2b = pool_set(list(reversed(seq2b)), KNOBS["u_pool_2b"])
            else:
                pool2a = {hk for hk in pool2 if hk[0] == 2 and hk[1] >= prefill}
                pool2b = {hk for hk in pool2 if hk[0] == 3}
                upool2a = {hk for hk in upool2 if hk[0] == 2 and hk[1] >= prefill}
                upool2b = {hk for hk in upool2 if hk[0] == 3}
            run_phase(seq2a, pool2a, upool2a, KNOBS["pool_la"])
            emit_finals(2, sts[2])
            run_phase(seq2b, pool2b, upool2b, KNOBS["pool_la"])
            emit_finals(3, sts[3], split_relu=True)

    nc.compile()
    return nc


def _get_nc():
    if "nc" not in _CACHE:
        _CACHE["nc"] = _build()
    return _CACHE["nc"]


def make_in_maps(inputs):
    X = np.ascontiguousarray(inputs["X"], dtype=np.float32)
    A = np.ascontiguousarray(inputs["A"], dtype=np.float32)
    W = np.ascontiguousarray(inputs["W"], dtype=np.float32)
    b = np.ascontiguousarray(inputs["b"], dtype=np.float32)
    a_self = np.ascontiguousarray(inputs["a_self"], dtype=np.float32)
    a_neigh = np.ascontiguousarray(inputs["a_neigh"], dtype=np.float32)
    # tiny host precompute: WA[h] = [W[h]@a_self[h] | W[h]@a_neigh[h]]
    WA = np.ascontiguousarray(
        np.stack([np.einsum("hfo,ho->hf", W, a_self),
                  np.einsum("hfo,ho->hf", W, a_neigh)], axis=2),
        dtype=np.float32)
    return [
        {
            "A": np.ascontiguousarray(A[i]),
            "X": np.ascontiguousarray(X[i]),
            "W": W,
            "b": b,
            "WA": WA,
        }
        for i in range(B)
    ]


def run(inputs, trace=False):
    from concourse import bass_utils

    nc = _get_nc()
    in_maps = make_in_maps(inputs)
    res = bass_utils.run_bass_kernel_spmd(
        nc, in_maps, core_ids=list(range(B)), trace=trace
    )
    out = np.empty((B, N, H * FH), dtype=np.float32)
    for i in range(B):
        o = np.asarray(res.results[i]["OUT"], dtype=np.float32)  # [H, N, FH]
        out[i] = o.transpose(1, 0, 2).reshape(N, H * FH)
    return out, res


def kernel(**inputs):
    out, _ = run(inputs, trace=False)
    return out


# revision 65
# speedup vs baseline: 1.0110x; 1.0007x over previous
"""Batch graph attention (GAT-style) Trainium2 kernel.

Problem: B=8, N=2048, F=64, FH=64, H=4.
  feats = X @ W[h]                         [B,H,N,FH]
  scores[n,m] = leaky_relu(s_self[n] + s_neigh[m], 0.2)
  P = softmax(scores + (1-A)*NEG_BIG, axis=m)
  out = relu(concat_h(P @ feats + b))

Sharding: batch b -> core b (8 cores, data parallel).

Per-core algorithm (neighbor index m on SBUF partitions):

  exp(leaky(x)) == max(e^x, e^{0.2x})  (slope<1); dropping the per-column
  factor e^{s_self[n]} (softmax columns are scale invariant) leaves

      Phat[m,n] = A^T[m,n] * max(e1[m], e2[m] * g[n])

  with e1=0.5*exp(s_neigh), e2=0.5*exp(0.2*s_neigh), g=exp(-0.8*s_self).
  Per (h,k-tile): u = (g_bc * e2) max e1 (DVE ts, 4x) and p = u * A^T
  (DVE tt, 2x).  Aggregation uses the TRANSPOSED matmul orientation:
  p-chunk [128m x 128n] is the PE stationary, G = [feats+b | 1] the
  65-col moving operand, accumulating agg[n, o] (+den at o=64) in PSUM
  per (head, ntile).  That makes den a per-partition column, so finals
  are: den cols -> SBUF (Act), one DVE reciprocal per head, and a
  per-ntile Act Relu(scale=1/den) straight out of PSUM.  Output leaves
  in natural [H, N, FH] orientation (host concatenates heads).

  A^T comes from fp32 A's fp16 bit-pair structure: fp16 view of fp32 1.0
  is [0x0000 | 0x3F80] = [0 | 1.875].  An xbar DMA transpose of 128 fp16
  columns starting at an ODD offset lands the 1.875*A values on EVEN
  output partitions (zeros on odd); the aligned window starting 128 later
  lands its values on ODD partitions.  A merge (DVE add, Pool add, or a
  partition-strided SBUF-to-SBUF DMA) produces dense 1.875*A^T with rows
  in the fixed interleave pi(p) = p/2 (p even) | 64+(p-1)/2 (p odd).
  The 1.875 cancels in the softmax; the pi permutation is absorbed by
  building XT16's columns pi-permuted, so G rows / e-vectors line up.
  g (an n-indexed row) is un-permuted during its PE transpose with the
  inverse identity.
"""

import numpy as np

B, N, F, FH, H = 8, 2048, 64, 64, 4
P = 128           # SBUF partitions
NT = N // P       # 16 m-tiles / n-tiles
C = 512           # chunk used for feats matmuls
NCH = N // C      # 4 chunks
GW = 66           # G row stride (64 feats + 1 ones + 1 pad)
LN_HALF = -0.6931471805599453

_CACHE = {}

# tuning knobs (read at build time)
KNOBS = {
    "tt_bufs": 3,         # xbar staging tile buffers
    "u_bufs": 14,
    "u_ahead": 10,         # emit u-ops this many seq steps ahead (needs u_bufs)
    "p_bufs": 5,
    "outp_bufs": 1,
    "lead": 2,            # merge lead (in k) ahead of consumption
    "pool_la": 4,         # lookahead (in seq steps) for pool-assigned ops
    "merge_dma_k": 16,    # merges with k >= this go via DMA
    "merge_pool": 0,      # first k merges on Pool (rest DVE)
    "prologue_merges": 5, # merges emitted before the first k-group
    "p_pool_1": 3,        # phase-1: every n-th (h,k) p-op on Pool
    "p_pool_2": 5,        # phase-2: every n-th (h,k) p-op on Pool
    "u_pool_1": 10,        # phase-1: every n-th (h,k) u-op on Pool
    "u_pool_2": 7,        # phase-2: every n-th (h,k) u-op on Pool
    "pool2_split": 0,     # independent per-sub-phase pool strides below
    "p_pool_2b": 3,
    "u_pool_2b": 4,
    "gbc_dma": True,      # g broadcast via DRAM bounce (else Pool)
    "gbc_pool_heads": (0, 1, 2),  # heads whose g broadcast goes on Pool
    "gbc_pe_heads": (),   # heads whose g broadcast goes via PE ones-matmul
    "fin_act": True,      # final relu/scale on Act (else DVE)
    "split_relu_at": 6,   # last head: relus with t >= this go on DVE
    "setup2_k": 14,        # phase-1 k at which head-2 setup is emitted
    "setup3_k": 15,       # phase-1 k at which head-3 setup is emitted
    "pe_warmup": 0,       # junk PE transposes for p-state ramp (0=off)
    "early_pairs": 0,     # merge tiles whose transposes precede X/W loads
    "half_merge_k": 3,    # first k merges done in column halves
    "quarter_merge_k": 0, # first k merges done in column quarters
    "merge_split_pool": 0,  # merges k in [2,2+n): lo half DVE, hi half Pool
    "prefill": 0,         # head-2 k-groups pre-built during phase-1 tail
    "act_split": 0,       # leading columns of u computed on Act (Prelu+Exp)
}

# agg PSUM bank grouping: ntiles per PSUM tile (65 f32 cols each; 7*65*4B
# = 1820B fits one 2KB bank, so slices never cross a bank boundary)
AGG_GROUPS = (7, 7, 2)


def _build():
    import concourse.bacc as bacc
    import concourse.tile as tile
    import concourse.mybir as mybir
    from concourse.mybir import AluOpType as op, ActivationFunctionType as act

    f32 = mybir.dt.float32
    fp16 = mybir.dt.float16
    i32 = mybir.dt.int32

    nc = bacc.Bacc(
        "TRN2",
        target_bir_lowering=False,
        debug=False,
        enable_asserts=False,
        num_devices=8,
    )

    A_d = nc.dram_tensor("A", [N, N], f32, kind="ExternalInput").ap()
    X_d = nc.dram_tensor("X", [N, F], f32, kind="ExternalInput").ap()
    W_d = nc.dram_tensor("W", [H, F, FH], f32, kind="ExternalInput").ap()
    b_d = nc.dram_tensor("b", [H, FH], f32, kind="ExternalInput").ap()
    # WA[h] = [W[h] @ a_self[h] | W[h] @ a_neigh[h]]  (tiny, host-precomputed)
    wa_d = nc.dram_tensor("WA", [H, F, 2], f32, kind="ExternalInput").ap()
    OUT_d = nc.dram_tensor("OUT", [H, N, FH], fp16, kind="ExternalOutput").ap()
    GROW_d = nc.dram_tensor("GROW", [H, N], fp16, kind="Internal").ap()
    SSROW_d = nc.dram_tensor("SSROW", [H, N], fp16, kind="Internal").ap()

    with tile.TileContext(nc) as tc:
        with (
            tc.tile_pool(name="const", bufs=1) as const,
            tc.tile_pool(name="big", bufs=1) as big,
            tc.tile_pool(name="stream", bufs=3) as stream,
            tc.tile_pool(name="head", bufs=2) as head,
            tc.tile_pool(name="outp", bufs=KNOBS["outp_bufs"]) as outp,
            tc.tile_pool(name="psu", bufs=2, space="PSUM") as psu,
            tc.tile_pool(name="psagg", bufs=2, space="PSUM") as psagg,
        ):
            # ---- constants --------------------------------------------
            iota_i = stream.tile([P, P], i32, tag="cst", bufs=2)
            nc.gpsimd.iota(iota_i[:], pattern=[[1, P]], base=0, channel_multiplier=0)
            pidx_i = stream.tile([P, 1], i32, tag="cst1", bufs=1)
            nc.gpsimd.iota(pidx_i[:], pattern=[[0, 1]], base=0, channel_multiplier=1)
            pidx_f = const.tile([P, 1], f32)
            nc.vector.tensor_copy(pidx_f[:], pidx_i[:])
            iota_f = stream.tile([P, P], f32, tag="cstf", bufs=2)
            nc.vector.tensor_copy(iota_f[:], iota_i[:])
            # pi sequence [0,64,1,65,...]: ident_pi[p,c]=1 iff p==pi(c)
            iopi_i = stream.tile([P, P], i32, tag="cst", bufs=2)
            nc.gpsimd.iota(iopi_i[:], pattern=[[1, 64], [64, 2]], base=0,
                           channel_multiplier=0)
            iopi_f = stream.tile([P, P], f32, tag="cstf", bufs=2)
            nc.vector.tensor_copy(iopi_f[:], iopi_i[:])
            ident_pi = const.tile([P, P], fp16)
            nc.vector.tensor_scalar(ident_pi[:], iopi_f[:], pidx_f[:], None,
                                    op.is_equal)
            # pinv sequence [0,2,..126,1,3,..127]: ident_pinv[p,c]=1 iff p==pinv(c)
            iopv_i = stream.tile([P, P], i32, tag="cst", bufs=2)
            nc.gpsimd.iota(iopv_i[:], pattern=[[1, 2], [2, 64]], base=0,
                           channel_multiplier=0)
            iopv_f = stream.tile([P, P], f32, tag="cstf", bufs=2)
            nc.vector.tensor_copy(iopv_f[:], iopv_i[:])
            ident_pinv = const.tile([P, P], fp16)
            nc.vector.tensor_scalar(ident_pinv[:], iopv_f[:], pidx_f[:], None,
                                    op.is_equal)

            lnhalf = const.tile([P, 1], f32)
            nc.vector.memset(lnhalf[:], LN_HALF)
            alpha02 = const.tile([P, 1], f32)
            nc.vector.memset(alpha02[:], 0.2)
            ones1 = const.tile([1, P], fp16)
            nc.vector.memset(ones1[:], 1.0)

            # PE pstate warmup: ~3us of junk transposes so the real early
            # matmuls run at full clock instead of the cold p-state.
            if KNOBS["pe_warmup"]:
                psW = psu.tile([P, P], fp16, tag="ps", name="pewarm")
                for _w in range(KNOBS["pe_warmup"]):
                    nc.tensor.transpose(psW[:], ident_pi[:], ident_pi[:])

            # ---- A^T via odd/even-offset fp16 xbar transposes + 1 merge ----
            AT_sb = big.tile([P, NT * N], fp16)
            Vf = A_d.bitcast(fp16)  # [2048, 4096]

            tb_store = {}

            def emit_merge_dmas(k):
                # odd-offset window transposed straight into AT_sb (values on
                # even partitions, zeros on odd); tb staged for the merge.
                # Early k-tiles split into row-halves so the first merge work
                # lands on DVE ~2x sooner.
                dst = AT_sb[:, k * N : (k + 1) * N]
                tb = stream.tile([P, N], fp16, tag="tt",
                                 bufs=KNOBS["tt_bufs"], name=f"tb_{k}")
                if k < KNOBS["quarter_merge_k"]:
                    for q in range(4):
                        lo, hi = q * N // 4, (q + 1) * N // 4
                        nc.sync.dma_start_transpose(
                            dst[:, lo:hi], Vf[lo:hi, 256 * k + 1 : 256 * k + 129])
                        nc.sync.dma_start_transpose(
                            tb[:, lo:hi],
                            Vf[lo:hi, 256 * k + 128 : 256 * k + 256])
                elif k < KNOBS["half_merge_k"]:
                    for lo, hi in ((0, N // 2), (N // 2, N)):
                        nc.sync.dma_start_transpose(
                            dst[:, lo:hi], Vf[lo:hi, 256 * k + 1 : 256 * k + 129])
                        nc.sync.dma_start_transpose(
                            tb[:, lo:hi],
                            Vf[lo:hi, 256 * k + 128 : 256 * k + 256])
                else:
                    nc.sync.dma_start_transpose(
                        dst, Vf[:, 256 * k + 1 : 256 * k + 129])
                    nc.sync.dma_start_transpose(
                        tb[:], Vf[:, 256 * k + 128 : 256 * k + 256])
                tb_store[k] = tb

            def emit_merge_tile(k):
                use_dma = k >= KNOBS["merge_dma_k"]
                use_pool = (not use_dma) and k < KNOBS["merge_pool"]
                dst = AT_sb[:, k * N : (k + 1) * N]
                if k not in tb_store:
                    emit_merge_dmas(k)
                tb = tb_store.pop(k)
                if use_dma:
                    nc.sync.dma_start(
                        AT_sb[:][1:P:2, k * N : (k + 1) * N], tb[:][1:P:2, :])
                elif k < KNOBS["quarter_merge_k"]:
                    eng = nc.gpsimd if use_pool else nc.vector
                    for q in range(4):
                        lo, hi = q * N // 4, (q + 1) * N // 4
                        eng.tensor_tensor(dst[:, lo:hi], dst[:, lo:hi],
                                          tb[:, lo:hi], op.add)
                elif k < KNOBS["half_merge_k"]:
                    eng = nc.gpsimd if use_pool else nc.vector
                    for lo, hi in ((0, N // 2), (N // 2, N)):
                        eng.tensor_tensor(dst[:, lo:hi], dst[:, lo:hi],
                                          tb[:, lo:hi], op.add)
                elif 2 <= k < 2 + KNOBS["merge_split_pool"]:
                    # lo half on DVE, hi half on Pool: sheds merge work to
                    # the Pool engine at half granularity
                    nc.vector.tensor_tensor(dst[:, 0 : N // 2],
                                            dst[:, 0 : N // 2],
                                            tb[:, 0 : N // 2], op.add)
                    nc.gpsimd.tensor_tensor(dst[:, N // 2 : N],
                                            dst[:, N // 2 : N],
                                            tb[:, N // 2 : N], op.add)
                else:
                    eng = nc.gpsimd if use_pool else nc.vector
                    eng.tensor_tensor(dst, dst, tb[:], op.add)

            merged = [0]

            def emit_merges_to(kmax):
                # merges go AFTER each k-group's u/p in the DVE stream: by
                # the time DVE reaches merge k+lead its xbar transposes have
                # long landed, so no head-of-line stall.
                while merged[0] < min(kmax, NT):
                    emit_merge_tile(merged[0])
                    merged[0] += 1

            # WA as fp16 [64, 2H]: wa16[:, 2h:2h+2] = W[h]@[a_self|a_neigh]
            avf = stream.tile([F, 2 * H], f32, tag="avf", bufs=1)
            nc.sync.dma_start(avf.rearrange("f (h two) -> f h two", two=2),
                              wa_d.rearrange("h f two -> f h two"))
            wa16 = const.tile([F, 2 * H], fp16)
            nc.vector.tensor_copy(wa16[:], avf[:])

            # ---- X -> XT16 [65, 2048] (fp16, pi-permuted cols, ones row 64)
            xf = stream.tile([P, NT * F], f32, tag="xf", bufs=1)
            nc.sync.dma_start(
                xf.rearrange("p (t f) -> p t f", f=F),
                X_d.rearrange("(t p) f -> p t f", p=P),
            )
            # first transpose pair(s) directly after the X load: the A xbar
            # stream owns the DMA engines early without delaying the g-chain
            for _k in range(KNOBS["early_pairs"]):
                emit_merge_dmas(_k)
            x16 = stream.tile([P, NT * F], fp16, tag="x16", bufs=1)
            nc.vector.tensor_copy(x16[:], xf[:])
            XT16 = big.tile([F + 1, N], fp16)
            for hx in range(2):
                xTps = psu.tile([F, N // 2], fp16, tag="ps", name=f"xTps_{hx}")
                for j in range(NT // 2):
                    t = hx * (NT // 2) + j
                    nc.tensor.transpose(
                        xTps[:, j * P : (j + 1) * P],
                        x16[:, t * F : (t + 1) * F],
                        ident_pi[:],
                    )
                nc.scalar.copy(XT16[0:F, hx * (N // 2) : (hx + 1) * (N // 2)],
                               xTps[:])
            nc.vector.memset(XT16[F : F + 1, :], 1.0)

            def emit_setup(h):
                Wf = head.tile([F + 1, FH], f32, tag="Wf", bufs=2,
                               name=f"Wf_{h}")
                nc.sync.dma_start(Wf[0:F, :], W_d[h])
                nc.sync.dma_start(Wf[F : F + 1, :], b_d[h : h + 1, :])
                W16 = head.tile([F + 1, FH], fp16, tag="W16", bufs=2,
                                name=f"W16_{h}")
                nc.vector.tensor_copy(W16[:], Wf[:])

                # s_self/s_neigh straight from XT16: s_j[n] = X[n,:]@(W a_j)
                psNg = psu.tile([P, 2 * NT], f32, tag="ps", name=f"psNg_{h}")
                psNg3 = psNg.rearrange("p (k two) -> p k two", two=2)
                for k in range(NT):
                    nc.tensor.matmul(
                        psNg3[:, k, :],
                        XT16[0:F, k * P : (k + 1) * P],
                        wa16[:, 2 * h : 2 * h + 2],
                        start=True, stop=True,
                    )
                # e1 = 0.5*exp(s_neigh), e2 = 0.5*exp(0.2*s_neigh)  (pi rows)
                e1g = head.tile([P, NT], f32, tag="e1g", bufs=4, name=f"e1g_{h}")
                nc.scalar.activation(e1g[:], psNg3[:, :, 1], act.Exp,
                                     scale=1.0, bias=lnhalf[:])
                e2g = head.tile([P, NT], f32, tag="e2g", bufs=4, name=f"e2g_{h}")
                nc.scalar.activation(e2g[:], psNg3[:, :, 1], act.Exp,
                                     scale=0.2, bias=lnhalf[:])
                ssg = head.tile([P, NT], fp16, tag="ssg", bufs=2, name=f"ssg_{h}")
                nc.scalar.copy(ssg[:], psNg3[:, :, 0])
                S = KNOBS["act_split"]
                snb = None
                if S:
                    snb = head.tile([P, NT], f32, tag="snb", bufs=2,
                                    name=f"snb_{h}")
                    nc.scalar.copy(snb[:], psNg3[:, :, 1])

                # g_row natural order: un-permute ssg with ident_pinv
                g_row = head.tile([1, N], fp16, tag="g_row", bufs=2,
                                  name=f"g_row_{h}")
                ss_row = (head.tile([1, S], fp16, tag="ss_row", bufs=2,
                                    name=f"ss_row_{h}") if S else None)
                for c in range(NCH):
                    psRow = psu.tile([1, C], fp16, tag="ps", name=f"psRow_{h}_{c}")
                    for j in range(4):
                        kk = c * 4 + j
                        nc.tensor.transpose(
                            psRow[:, j * P : (j + 1) * P],
                            ssg[:, kk : kk + 1],
                            ident_pinv[:],
                        )
                    nc.scalar.activation(
                        g_row[:, c * C : (c + 1) * C], psRow[:], act.Exp,
                        scale=-0.8,
                    )
                    if S and c * C < S:
                        w = min(C, S - c * C)
                        nc.scalar.copy(ss_row[:, c * C : c * C + w],
                                       psRow[:, 0:w])
                g_bc = head.tile([P, N], fp16, tag="g_bc", bufs=4, name=f"g_bc_{h}")
                if h in KNOBS["gbc_pe_heads"]:
                    # ones-matmul broadcast: g_row streams through the PE and
                    # lands replicated on all partitions (f32 PSUM chunks),
                    # then DVE casts to fp16
                    for c in range(NCH):
                        psB = psu.tile([P, C], f32, tag="ps", name=f"psB_{h}_{c}")
                        nc.tensor.matmul(psB[:], ones1[:],
                                         g_row[:, c * C : (c + 1) * C],
                                         start=True, stop=True)
                        nc.vector.tensor_copy(g_bc[:, c * C : (c + 1) * C],
                                              psB[:])
                elif KNOBS["gbc_dma"] and h not in KNOBS["gbc_pool_heads"]:
                    nc.scalar.dma_start(GROW_d[h : h + 1, :], g_row[:])
                    nc.scalar.dma_start(
                        g_bc[:],
                        GROW_d[h : h + 1, :].partition_broadcast(P).squeeze(1))
                else:
                    nc.gpsimd.partition_broadcast(g_bc[:], g_row[:])

                ss_bc = None
                if S:
                    ss_bc = head.tile([P, S], fp16, tag="ss_bc", bufs=4,
                                      name=f"ss_bc_{h}")
                    nc.scalar.dma_start(SSROW_d[h : h + 1, 0:S], ss_row[:])
                    nc.scalar.dma_start(
                        ss_bc[:],
                        SSROW_d[h : h + 1, 0:S].partition_broadcast(P).squeeze(1))

                G_all = head.tile([P, NT * GW], fp16, tag="G_all", bufs=4,
                                  name=f"G_all_{h}")
                G3 = G_all.rearrange("p (k w) -> p k w", w=GW)
                for halfg in range(2):
                    psG = psu.tile([P, (NT // 2) * FH], f32, tag="ps",
                                   name=f"psG_{h}_{halfg}")
                    for j in range(NT // 2):
                        k = halfg * (NT // 2) + j
                        nc.tensor.matmul(
                            psG[:, j * FH : (j + 1) * FH],
                            XT16[:, k * P : (k + 1) * P],
                            W16[:],
                            start=True, stop=True,
                        )
                    nc.scalar.copy(
                        G3[:, halfg * (NT // 2) : (halfg + 1) * (NT // 2), 0:FH],
                        psG.rearrange("p (k f) -> p k f", f=FH),
                    )
                nc.vector.memset(G3[:, :, FH : FH + 1], 1.0)
                return {"e1g": e1g, "e2g": e2g, "g_bc": g_bc,
                        "G_all": G_all, "agg": None,
                        "snb": snb, "ss_bc": ss_bc}

            def emit_u(h, st, k, use_pool):
                e1g, e2g, g_bc = st["e1g"], st["e2g"], st["g_bc"]
                u_t = stream.tile([P, N], fp16, tag="u", bufs=KNOBS["u_bufs"],
                                  name=f"u_{h}_{k}")
                S = KNOBS["act_split"]
                eng = nc.gpsimd if use_pool else nc.vector
                if S and not use_pool:
                    # columns [0:S] exactly on Act: exp(leaky(ss+sn)); the
                    # per-column softmax factor differs from the DVE half's
                    # convention but cancels in the normalization.
                    tmp = stream.tile([P, S], fp16, tag="uact", bufs=3,
                                      name=f"ua_{h}_{k}")
                    nc.scalar.activation(tmp[:], st["ss_bc"][:], act.Prelu,
                                         bias=st["snb"][:, k : k + 1],
                                         scale=1.0, alpha=alpha02[:])
                    nc.scalar.activation(u_t[:, 0:S], tmp[:], act.Exp)
                    eng.tensor_scalar(
                        u_t[:, S:N], g_bc[:, S:N],
                        e2g[:, k : k + 1], e1g[:, k : k + 1],
                        op.mult, op.max,
                    )
                else:
                    eng.tensor_scalar(
                        u_t[:], g_bc[:],
                        e2g[:, k : k + 1], e1g[:, k : k + 1],
                        op.mult, op.max,
                    )
                return u_t

            def emit_p(h, st, k, u_t, use_pool, tag="p", bufs=None):
                p_t = stream.tile([P, N], fp16, tag=tag,
                                  bufs=bufs or KNOBS["p_bufs"],
                                  name=f"p_{h}_{k}")
                eng = nc.gpsimd if use_pool else nc.vector
                eng.tensor_tensor(
                    p_t[:], u_t[:], AT_sb[:, k * N : (k + 1) * N], op.mult
                )
                return p_t

            def emit_aggs(h, st, k, p_t):
                G_all = st["G_all"]
                if st["agg"] is None:
                    st["agg"] = [
                        psagg.tile([P, g * 65], f32, tag=f"agg{gi}",
                                   name=f"agg{h}_{gi}")
                        for gi, g in enumerate(AGG_GROUPS)
                    ]
                aggs = st["agg"]
                rhs = st["G_all"][:, k * GW : k * GW + 65]
                # PSUM start=True lazily zeroes the whole 2KB bank, so only
                # the FIRST matmul of each bank-group tile may set it; later
                # slices overwrite their pending-zero bytes with start=False.
                for t in range(NT):
                    gi = 0 if t < 7 else (1 if t < 14 else 2)
                    tt_ = t - (0 if t < 7 else (7 if t < 14 else 14))
                    last = AGG_GROUPS[gi] - 1
                    nc.tensor.matmul(
                        aggs[gi][:, tt_ * 65 : tt_ * 65 + 65],
                        p_t[:, t * P : (t + 1) * P],
                        rhs,
                        start=(k == 0 and tt_ == 0),
                        stop=(k == NT - 1 and tt_ == last),
                    )

            def emit_finals(h, st, split_relu=False):
                aggs = st["agg"]
                # den columns (o=64 of each 65-group) -> SBUF, then 1/den
                den = head.tile([P, NT], f32, tag="den", bufs=2,
                                name=f"den_{h}")
                base = 0
                for gi, g in enumerate(AGG_GROUPS):
                    a3 = aggs[gi].rearrange("p (t w) -> p t w", w=65)
                    nc.scalar.copy(den[:, base : base + g], a3[:, :, 64])
                    base += g
                r = head.tile([P, NT], f32, tag="r", bufs=2, name=f"r_{h}")
                nc.vector.reciprocal_approx_fast(r[:], den[:])
                out_sb = outp.tile([P, NT * FH], fp16, tag="outf",
                                   name=f"outf_{h}")
                base = 0
                for gi, g in enumerate(AGG_GROUPS):
                    a3 = aggs[gi].rearrange("p (t w) -> p t w", w=65)
                    for j in range(g):
                        t = base + j
                        if KNOBS["fin_act"] and not (split_relu and t >= KNOBS["split_relu_at"]):
                            nc.scalar.activation(
                                out_sb[:, t * FH : (t + 1) * FH],
                                a3[:, j, 0:FH], act.Relu,
                                scale=r[:, t : t + 1],
                            )
                        else:
                            nc.vector.tensor_scalar(
                                out_sb[:, t * FH : (t + 1) * FH],
                                a3[:, j, 0:FH],
                                r[:, t : t + 1], 0.0, op.mult, op.max,
                            )
                    base += g
                if split_relu:
                    # two half DMAs so the first overlaps the remaining relus
                    ht = NT // 2
                    nc.sync.dma_start(
                        OUT_d[h, 0 : ht * P].rearrange("(t p) f -> p t f", p=P),
                        out_sb.rearrange("p (t f) -> p t f", f=FH)[:, 0:ht, :],
                    )
                    nc.sync.dma_start(
                        OUT_d[h, ht * P : N].rearrange("(t p) f -> p t f", p=P),
                        out_sb.rearrange("p (t f) -> p t f", f=FH)[:, ht:NT, :],
                    )
                else:
                    nc.sync.dma_start(
                        OUT_d[h].rearrange("(t p) f -> p t f", p=P),
                        out_sb.rearrange("p (t f) -> p t f", f=FH),
                    )

            # ---- schedule ------------------------------------------------
            # all four setups run before aggs; heads 0/1 aggregate while A^T
            # streams in, heads 2/3 afterwards.  Pool-assigned u/p ops are
            # emitted with lookahead so the in-order PE agg queue never waits
            # on the slower Pool engine.
            sts = [emit_setup(0), emit_setup(1), None, None]
            seq1 = [(h, k) for k in range(NT) for h in (0, 1)]
            seq2 = [(h, k) for k in range(NT) for h in (2, 3)]

            def pool_set(seq, stride):
                if not stride:
                    return set()
                return {hk for i, hk in enumerate(seq) if i % stride == stride - 1}

            def run_phase(seq, p_pool, u_pool, la, per_k=None, post_k=None,
                          group_done=None):
                pend = {}
                ustore = {}
                emitted = set()
                uahead = KNOBS["u_ahead"]

                def get_u(idx):
                    if idx in ustore:
                        return ustore.pop(idx)
                    h, k = seq[idx]
                    return emit_u(h, sts[h], k, seq[idx] in u_pool)

                def produce_u(idx):
                    # u only needs g_bc: emit ahead so DVE has filler work
                    # while waiting on the A^T merge stream
                    if idx >= len(seq) or idx in ustore or seq[idx] in emitted:
                        return
                    h, k = seq[idx]
                    if sts[h] is None:
                        return
                    ustore[idx] = emit_u(h, sts[h], k, seq[idx] in u_pool)

                def produce(idx):
                    if idx >= len(seq) or seq[idx] in emitted:
                        return
                    h, k = seq[idx]
                    if seq[idx] in p_pool or seq[idx] in u_pool:
                        emitted.add(seq[idx])
                        pend[(h, k)] = emit_p(h, sts[h], k, get_u(idx),
                                              seq[idx] in p_pool)

                lastk = -1
                for i, (h, k) in enumerate(seq):
                    if k != lastk:
                        if per_k is not None:
                            per_k(k)
                        if post_k is not None and lastk >= 0:
                            post_k(lastk)
                        lastk = k
                    for j in range(i, min(i + uahead + 1, len(seq))):
                        produce_u(j)
                    for j in range(i, min(i + la + 1, len(seq))):
                        produce(j)
                    if (h, k) in pend:
                        emit_aggs(h, sts[h], k, pend.pop((h, k)))
                    else:
                        emit_aggs(h, sts[h], k,
                                  emit_p(h, sts[h], k, get_u(i), False))
                    if group_done is not None and (i + 1 == len(seq)
                                                   or seq[i + 1][1] != k):
                        group_done(k)
                if post_k is not None and lastk >= 0:
                    post_k(lastk)

            lead = KNOBS["lead"]

            def per_k1(k):
                if k == 0:
                    emit_merges_to(KNOBS["prologue_merges"])
                if k == KNOBS["setup2_k"]:
                    sts[2] = emit_setup(2)
                if k == KNOBS["setup3_k"]:
                    sts[3] = emit_setup(3)

            def post_k1(k):
                emit_merges_to(k + lead + 2)

            pool1 = pool_set(seq1, KNOBS["p_pool_1"])
            pool2 = pool_set(seq2, KNOBS["p_pool_2"])
            upool1 = pool_set(list(reversed(seq1)), KNOBS["u_pool_1"])
            upool2 = pool_set(list(reversed(seq2)), KNOBS["u_pool_2"])

            # prefill: during phase-1 tail, pre-build head-2 u/p for early k
            # (aggs deferred until its PSUM frees after finals(0)/finals(1))
            prefill = KNOBS["prefill"]
            p2_store = {}

            def group_done1(k):
                j = k - (NT - prefill)
                if 0 <= j < prefill and sts[2] is not None:
                    u2 = emit_u(2, sts[2], j, False)
                    p2_store[j] = emit_p(2, sts[2], j, u2, False,
                                         tag="p2", bufs=max(prefill, 1))

            run_phase(seq1, pool1, upool1, KNOBS["pool_la"], per_k=per_k1,
                      post_k=post_k1, group_done=group_done1)
            emit_finals(0, sts[0])
            emit_finals(1, sts[1])
            seq2a = [(2, k) for k in range(NT) if k not in p2_store]
            seq2b = [(3, k) for k in range(NT)]
            for j in sorted(p2_store):
                emit_aggs(2, sts[2], j, p2_store.pop(j))
            if KNOBS["pool2_split"]:
                # independent pool strides per sub-phase (head 2 vs head 3)
                pool2a = pool_set(seq2a, KNOBS["p_pool_2"])
                pool2b = pool_set(seq2b, KNOBS["p_pool_2b"])
                upool2a = pool_set(list(reversed(seq2a)), KNOBS["u_pool_2"])
                upool2b = pool_set(list(reversed(seq2b)), KNOBS["u_pool_2b"])
            else:
                pool2a = {hk for hk in pool2 if hk[0] == 2 and hk[1] >= prefill}
                pool2b = {hk for hk in pool2 if hk[0] == 3}
                upool2a = {hk for hk in upool2 if hk[0] == 2 and hk[1] >= prefill}
                upool2b = {hk for hk in upool2 if hk[0] == 3}
            run_phase(seq2a, pool2a, upool2a, KNOBS["pool_la"])
            emit_finals(2, sts[2])
            run_phase(seq2b, pool2b, upool2b, KNOBS["pool_la"])
            emit_finals(3, sts[3], split_relu=True)

    nc.compile()
    return nc


def _get_nc():
    if "nc" not in _CACHE:
        _CACHE["nc"] = _build()
    return _CACHE["nc"]


def make_in_maps(inputs):
    X = np.ascontiguousarray(inputs["X"], dtype=np.float32)
    A = np.ascontiguousarray(inputs["A"], dtype=np.float32)
    W = np.ascontiguousarray(inputs["W"], dtype=np.float32)
    b = np.ascontiguousarray(inputs["b"], dtype=np.float32)
    a_self = np.ascontiguousarray(inputs["a_self"], dtype=np.float32)
    a_neigh = np.ascontiguousarray(inputs["a_neigh"], dtype=np.float32)
    # tiny host precompute: WA[h] = [W[h]@a_self[h] | W[h]@a_neigh[h]]
    WA = np.ascontiguousarray(
        np.stack([np.einsum("hfo,ho->hf", W, a_self),
                  np.einsum("hfo,ho->hf", W, a_neigh)], axis=2),
        dtype=np.float32)
    return [
        {
            "A": np.ascontiguousarray(A[i]),
            "X": np.ascontiguousarray(X[i]),
            "W": W,
            "b": b,
            "WA": WA,
        }
        for i in range(B)
    ]


def run(inputs, trace=False):
    from concourse import bass_utils

    nc = _get_nc()
    in_maps = make_in_maps(inputs)
    res = bass_utils.run_bass_kernel_spmd(
        nc, in_maps, core_ids=list(range(B)), trace=trace
    )
    out = np.empty((B, N, H * FH), dtype=np.float32)
    for i in range(B):
        o = np.asarray(res.results[i]["OUT"], dtype=np.float32)  # [H, N, FH]
        out[i] = o.transpose(1, 0, 2).reshape(N, H * FH)
    return out, res


def kernel(**inputs):
    out, _ = run(inputs, trace=False)
    return out


# revision 66
# speedup vs baseline: 1.0127x; 1.0017x over previous
"""Batch graph attention (GAT-style) Trainium2 kernel.

Problem: B=8, N=2048, F=64, FH=64, H=4.
  feats = X @ W[h]                         [B,H,N,FH]
  scores[n,m] = leaky_relu(s_self[n] + s_neigh[m], 0.2)
  P = softmax(scores + (1-A)*NEG_BIG, axis=m)
  out = relu(concat_h(P @ feats + b))

Sharding: batch b -> core b (8 cores, data parallel).

Per-core algorithm (neighbor index m on SBUF partitions):

  exp(leaky(x)) == max(e^x, e^{0.2x})  (slope<1); dropping the per-column
  factor e^{s_self[n]} (softmax columns are scale invariant) leaves

      Phat[m,n] = A^T[m,n] * max(e1[m], e2[m] * g[n])

  with e1=0.5*exp(s_neigh), e2=0.5*exp(0.2*s_neigh), g=exp(-0.8*s_self).
  Per (h,k-tile): u = (g_bc * e2) max e1 (DVE ts, 4x) and p = u * A^T
  (DVE tt, 2x).  Aggregation uses the TRANSPOSED matmul orientation:
  p-chunk [128m x 128n] is the PE stationary, G = [feats+b | 1] the
  65-col moving operand, accumulating agg[n, o] (+den at o=64) in PSUM
  per (head, ntile).  That makes den a per-partition column, so finals
  are: den cols -> SBUF (Act), one DVE reciprocal per head, and a
  per-ntile Act Relu(scale=1/den) straight out of PSUM.  Output leaves
  in natural [H, N, FH] orientation (host concatenates heads).

  A^T comes from fp32 A's fp16 bit-pair structure: fp16 view of fp32 1.0
  is [0x0000 | 0x3F80] = [0 | 1.875].  An xbar DMA transpose of 128 fp16
  columns starting at an ODD offset lands the 1.875*A values on EVEN
  output partitions (zeros on odd); the aligned window starting 128 later
  lands its values on ODD partitions.  A merge (DVE add, Pool add, or a
  partition-strided SBUF-to-SBUF DMA) produces dense 1.875*A^T with rows
  in the fixed interleave pi(p) = p/2 (p even) | 64+(p-1)/2 (p odd).
  The 1.875 cancels in the softmax; the pi permutation is absorbed by
  building XT16's columns pi-permuted, so G rows / e-vectors line up.
  g (an n-indexed row) is un-permuted during its PE transpose with the
  inverse identity.
"""

import numpy as np

B, N, F, FH, H = 8, 2048, 64, 64, 4
P = 128           # SBUF partitions
NT = N // P       # 16 m-tiles / n-tiles
C = 512           # chunk used for feats matmuls
NCH = N // C      # 4 chunks
GW = 66           # G row stride (64 feats + 1 ones + 1 pad)
LN_HALF = -0.6931471805599453

_CACHE = {}

# tuning knobs (read at build time)
KNOBS = {
    "tt_bufs": 3,         # xbar staging tile buffers
    "u_bufs": 14,
    "u_ahead": 10,         # emit u-ops this many seq steps ahead (needs u_bufs)
    "p_bufs": 5,
    "outp_bufs": 1,
    "lead": 2,            # merge lead (in k) ahead of consumption
    "pool_la": 4,         # lookahead (in seq steps) for pool-assigned ops
    "merge_dma_k": 16,    # merges with k >= this go via DMA
    "merge_pool": 0,      # first k merges on Pool (rest DVE)
    "prologue_merges": 5, # merges emitted before the first k-group
    "p_pool_1": 3,        # phase-1: every n-th (h,k) p-op on Pool
    "p_pool_2": 5,        # phase-2: every n-th (h,k) p-op on Pool
    "u_pool_1": 10,        # phase-1: every n-th (h,k) u-op on Pool
    "u_pool_2": 7,        # phase-2: every n-th (h,k) u-op on Pool
    "pool2_split": 0,     # independent per-sub-phase pool strides below
    "p_pool_2b": 3,
    "u_pool_2b": 4,
    "gbc_dma": True,      # g broadcast via DRAM bounce (else Pool)
    "gbc_pool_heads": (0, 1, 2),  # heads whose g broadcast goes on Pool
    "gbc_pe_heads": (),   # heads whose g broadcast goes via PE ones-matmul
    "fin_act": True,      # final relu/scale on Act (else DVE)
    "split_relu_at": 6,   # last head: relus with t >= this go on DVE
    "setup2_k": 14,        # phase-1 k at which head-2 setup is emitted
    "setup3_k": 15,       # phase-1 k at which head-3 setup is emitted
    "pe_warmup": 0,       # junk PE transposes for p-state ramp (0=off)
    "early_pairs": 0,     # merge tiles whose transposes precede X/W loads
    "half_merge_k": 4,    # first k merges done in column halves
    "quarter_merge_k": 0, # first k merges done in column quarters
    "merge_split_pool": 0,  # merges k in [2,2+n): lo half DVE, hi half Pool
    "prefill": 0,         # head-2 k-groups pre-built during phase-1 tail
    "act_split": 0,       # leading columns of u computed on Act (Prelu+Exp)
}

# agg PSUM bank grouping: ntiles per PSUM tile (65 f32 cols each; 7*65*4B
# = 1820B fits one 2KB bank, so slices never cross a bank boundary)
AGG_GROUPS = (7, 7, 2)


def _build():
    import concourse.bacc as bacc
    import concourse.tile as tile
    import concourse.mybir as mybir
    from concourse.mybir import AluOpType as op, ActivationFunctionType as act

    f32 = mybir.dt.float32
    fp16 = mybir.dt.float16
    i32 = mybir.dt.int32

    nc = bacc.Bacc(
        "TRN2",
        target_bir_lowering=False,
        debug=False,
        enable_asserts=False,
        num_devices=8,
    )

    A_d = nc.dram_tensor("A", [N, N], f32, kind="ExternalInput").ap()
    X_d = nc.dram_tensor("X", [N, F], f32, kind="ExternalInput").ap()
    W_d = nc.dram_tensor("W", [H, F, FH], f32, kind="ExternalInput").ap()
    b_d = nc.dram_tensor("b", [H, FH], f32, kind="ExternalInput").ap()
    # WA[h] = [W[h] @ a_self[h] | W[h] @ a_neigh[h]]  (tiny, host-precomputed)
    wa_d = nc.dram_tensor("WA", [H, F, 2], f32, kind="ExternalInput").ap()
    OUT_d = nc.dram_tensor("OUT", [H, N, FH], fp16, kind="ExternalOutput").ap()
    GROW_d = nc.dram_tensor("GROW", [H, N], fp16, kind="Internal").ap()
    SSROW_d = nc.dram_tensor("SSROW", [H, N], fp16, kind="Internal").ap()

    with tile.TileContext(nc) as tc:
        with (
            tc.tile_pool(name="const", bufs=1) as const,
            tc.tile_pool(name="big", bufs=1) as big,
            tc.tile_pool(name="stream", bufs=3) as stream,
            tc.tile_pool(name="head", bufs=2) as head,
            tc.tile_pool(name="outp", bufs=KNOBS["outp_bufs"]) as outp,
            tc.tile_pool(name="psu", bufs=2, space="PSUM") as psu,
            tc.tile_pool(name="psagg", bufs=2, space="PSUM") as psagg,
        ):
            # ---- constants --------------------------------------------
            iota_i = stream.tile([P, P], i32, tag="cst", bufs=2)
            nc.gpsimd.iota(iota_i[:], pattern=[[1, P]], base=0, channel_multiplier=0)
            pidx_i = stream.tile([P, 1], i32, tag="cst1", bufs=1)
            nc.gpsimd.iota(pidx_i[:], pattern=[[0, 1]], base=0, channel_multiplier=1)
            pidx_f = const.tile([P, 1], f32)
            nc.vector.tensor_copy(pidx_f[:], pidx_i[:])
            iota_f = stream.tile([P, P], f32, tag="cstf", bufs=2)
            nc.vector.tensor_copy(iota_f[:], iota_i[:])
            # pi sequence [0,64,1,65,...]: ident_pi[p,c]=1 iff p==pi(c)
            iopi_i = stream.tile([P, P], i32, tag="cst", bufs=2)
            nc.gpsimd.iota(iopi_i[:], pattern=[[1, 64], [64, 2]], base=0,
                           channel_multiplier=0)
            iopi_f = stream.tile([P, P], f32, tag="cstf", bufs=2)
            nc.vector.tensor_copy(iopi_f[:], iopi_i[:])
            ident_pi = const.tile([P, P], fp16)
            nc.vector.tensor_scalar(ident_pi[:], iopi_f[:], pidx_f[:], None,
                                    op.is_equal)
            # pinv sequence [0,2,..126,1,3,..127]: ident_pinv[p,c]=1 iff p==pinv(c)
            iopv_i = stream.tile([P, P], i32, tag="cst", bufs=2)
            nc.gpsimd.iota(iopv_i[:], pattern=[[1, 2], [2, 64]], base=0,
                           channel_multiplier=0)
            iopv_f = stream.tile([P, P], f32, tag="cstf", bufs=2)
            nc.vector.tensor_copy(iopv_f[:], iopv_i[:])
            ident_pinv = const.tile([P, P], fp16)
            nc.vector.tensor_scalar(ident_pinv[:], iopv_f[:], pidx_f[:], None,
                                    op.is_equal)

            lnhalf = const.tile([P, 1], f32)
            nc.vector.memset(lnhalf[:], LN_HALF)
            alpha02 = const.tile([P, 1], f32)
            nc.vector.memset(alpha02[:], 0.2)
            ones1 = const.tile([1, P], fp16)
            nc.vector.memset(ones1[:], 1.0)

            # PE pstate warmup: ~3us of junk transposes so the real early
            # matmuls run at full clock instead of the cold p-state.
            if KNOBS["pe_warmup"]:
                psW = psu.tile([P, P], fp16, tag="ps", name="pewarm")
                for _w in range(KNOBS["pe_warmup"]):
                    nc.tensor.transpose(psW[:], ident_pi[:], ident_pi[:])

            # ---- A^T via odd/even-offset fp16 xbar transposes + 1 merge ----
            AT_sb = big.tile([P, NT * N], fp16)
            Vf = A_d.bitcast(fp16)  # [2048, 4096]

            tb_store = {}

            def emit_merge_dmas(k):
                # odd-offset window transposed straight into AT_sb (values on
                # even partitions, zeros on odd); tb staged for the merge.
                # Early k-tiles split into row-halves so the first merge work
                # lands on DVE ~2x sooner.
                dst = AT_sb[:, k * N : (k + 1) * N]
                tb = stream.tile([P, N], fp16, tag="tt",
                                 bufs=KNOBS["tt_bufs"], name=f"tb_{k}")
                if k < KNOBS["quarter_merge_k"]:
                    for q in range(4):
                        lo, hi = q * N // 4, (q + 1) * N // 4
                        nc.sync.dma_start_transpose(
                            dst[:, lo:hi], Vf[lo:hi, 256 * k + 1 : 256 * k + 129])
                        nc.sync.dma_start_transpose(
                            tb[:, lo:hi],
                            Vf[lo:hi, 256 * k + 128 : 256 * k + 256])
                elif k < KNOBS["half_merge_k"]:
                    for lo, hi in ((0, N // 2), (N // 2, N)):
                        nc.sync.dma_start_transpose(
                            dst[:, lo:hi], Vf[lo:hi, 256 * k + 1 : 256 * k + 129])
                        nc.sync.dma_start_transpose(
                            tb[:, lo:hi],
                            Vf[lo:hi, 256 * k + 128 : 256 * k + 256])
                else:
                    nc.sync.dma_start_transpose(
                        dst, Vf[:, 256 * k + 1 : 256 * k + 129])
                    nc.sync.dma_start_transpose(
                        tb[:], Vf[:, 256 * k + 128 : 256 * k + 256])
                tb_store[k] = tb

            def emit_merge_tile(k):
                use_dma = k >= KNOBS["merge_dma_k"]
                use_pool = (not use_dma) and k < KNOBS["merge_pool"]
                dst = AT_sb[:, k * N : (k + 1) * N]
                if k not in tb_store:
                    emit_merge_dmas(k)
                tb = tb_store.pop(k)
                if use_dma:
                    nc.sync.dma_start(
                        AT_sb[:][1:P:2, k * N : (k + 1) * N], tb[:][1:P:2, :])
                elif k < KNOBS["quarter_merge_k"]:
                    eng = nc.gpsimd if use_pool else nc.vector
                    for q in range(4):
                        lo, hi = q * N // 4, (q + 1) * N // 4
                        eng.tensor_tensor(dst[:, lo:hi], dst[:, lo:hi],
                                          tb[:, lo:hi], op.add)
                elif k < KNOBS["half_merge_k"]:
                    eng = nc.gpsimd if use_pool else nc.vector
                    for lo, hi in ((0, N // 2), (N // 2, N)):
                        eng.tensor_tensor(dst[:, lo:hi], dst[:, lo:hi],
                                          tb[:, lo:hi], op.add)
                elif 2 <= k < 2 + KNOBS["merge_split_pool"]:
                    # lo half on DVE, hi half on Pool: sheds merge work to
                    # the Pool engine at half granularity
                    nc.vector.tensor_tensor(dst[:, 0 : N // 2],
                                            dst[:, 0 : N // 2],
                                            tb[:, 0 : N // 2], op.add)
                    nc.gpsimd.tensor_tensor(dst[:, N // 2 : N],
                                            dst[:, N // 2 : N],
                                            tb[:, N // 2 : N], op.add)
                else:
                    eng = nc.gpsimd if use_pool else nc.vector
                    eng.tensor_tensor(dst, dst, tb[:], op.add)

            merged = [0]

            def emit_merges_to(kmax):
                # merges go AFTER each k-group's u/p in the DVE stream: by
                # the time DVE reaches merge k+lead its xbar transposes have
                # long landed, so no head-of-line stall.
                while merged[0] < min(kmax, NT):
                    emit_merge_tile(merged[0])
                    merged[0] += 1

            # WA as fp16 [64, 2H]: wa16[:, 2h:2h+2] = W[h]@[a_self|a_neigh]
            avf = stream.tile([F, 2 * H], f32, tag="avf", bufs=1)
            nc.sync.dma_start(avf.rearrange("f (h two) -> f h two", two=2),
                              wa_d.rearrange("h f two -> f h two"))
            wa16 = const.tile([F, 2 * H], fp16)
            nc.vector.tensor_copy(wa16[:], avf[:])

            # ---- X -> XT16 [65, 2048] (fp16, pi-permuted cols, ones row 64)
            xf = stream.tile([P, NT * F], f32, tag="xf", bufs=1)
            nc.sync.dma_start(
                xf.rearrange("p (t f) -> p t f", f=F),
                X_d.rearrange("(t p) f -> p t f", p=P),
            )
            # first transpose pair(s) directly after the X load: the A xbar
            # stream owns the DMA engines early without delaying the g-chain
            for _k in range(KNOBS["early_pairs"]):
                emit_merge_dmas(_k)
            x16 = stream.tile([P, NT * F], fp16, tag="x16", bufs=1)
            nc.vector.tensor_copy(x16[:], xf[:])
            XT16 = big.tile([F + 1, N], fp16)
            for hx in range(2):
                xTps = psu.tile([F, N // 2], fp16, tag="ps", name=f"xTps_{hx}")
                for j in range(NT // 2):
                    t = hx * (NT // 2) + j
                    nc.tensor.transpose(
                        xTps[:, j * P : (j + 1) * P],
                        x16[:, t * F : (t + 1) * F],
                        ident_pi[:],
                    )
                nc.scalar.copy(XT16[0:F, hx * (N // 2) : (hx + 1) * (N // 2)],
                               xTps[:])
            nc.vector.memset(XT16[F : F + 1, :], 1.0)

            def emit_setup(h):
                Wf = head.tile([F + 1, FH], f32, tag="Wf", bufs=2,
                               name=f"Wf_{h}")
                nc.sync.dma_start(Wf[0:F, :], W_d[h])
                nc.sync.dma_start(Wf[F : F + 1, :], b_d[h : h + 1, :])
                W16 = head.tile([F + 1, FH], fp16, tag="W16", bufs=2,
                                name=f"W16_{h}")
                nc.vector.tensor_copy(W16[:], Wf[:])

                # s_self/s_neigh straight from XT16: s_j[n] = X[n,:]@(W a_j)
                psNg = psu.tile([P, 2 * NT], f32, tag="ps", name=f"psNg_{h}")
                psNg3 = psNg.rearrange("p (k two) -> p k two", two=2)
                for k in range(NT):
                    nc.tensor.matmul(
                        psNg3[:, k, :],
                        XT16[0:F, k * P : (k + 1) * P],
                        wa16[:, 2 * h : 2 * h + 2],
                        start=True, stop=True,
                    )
                # e1 = 0.5*exp(s_neigh), e2 = 0.5*exp(0.2*s_neigh)  (pi rows)
                e1g = head.tile([P, NT], f32, tag="e1g", bufs=4, name=f"e1g_{h}")
                nc.scalar.activation(e1g[:], psNg3[:, :, 1], act.Exp,
                                     scale=1.0, bias=lnhalf[:])
                e2g = head.tile([P, NT], f32, tag="e2g", bufs=4, name=f"e2g_{h}")
                nc.scalar.activation(e2g[:], psNg3[:, :, 1], act.Exp,
                                     scale=0.2, bias=lnhalf[:])
                ssg = head.tile([P, NT], fp16, tag="ssg", bufs=2, name=f"ssg_{h}")
                nc.scalar.copy(ssg[:], psNg3[:, :, 0])
                S = KNOBS["act_split"]
                snb = None
                if S:
                    snb = head.tile([P, NT], f32, tag="snb", bufs=2,
                                    name=f"snb_{h}")
                    nc.scalar.copy(snb[:], psNg3[:, :, 1])

                # g_row natural order: un-permute ssg with ident_pinv
                g_row = head.tile([1, N], fp16, tag="g_row", bufs=2,
                                  name=f"g_row_{h}")
                ss_row = (head.tile([1, S], fp16, tag="ss_row", bufs=2,
                                    name=f"ss_row_{h}") if S else None)
                for c in range(NCH):
                    psRow = psu.tile([1, C], fp16, tag="ps", name=f"psRow_{h}_{c}")
                    for j in range(4):
                        kk = c * 4 + j
                        nc.tensor.transpose(
                            psRow[:, j * P : (j + 1) * P],
                            ssg[:, kk : kk + 1],
                            ident_pinv[:],
                        )
                    nc.scalar.activation(
                        g_row[:, c * C : (c + 1) * C], psRow[:], act.Exp,
                        scale=-0.8,
                    )
                    if S and c * C < S:
                        w = min(C, S - c * C)
                        nc.scalar.copy(ss_row[:, c * C : c * C + w],
                                       psRow[:, 0:w])
                g_bc = head.tile([P, N], fp16, tag="g_bc", bufs=4, name=f"g_bc_{h}")
                if h in KNOBS["gbc_pe_heads"]:
                    # ones-matmul broadcast: g_row streams through the PE and
                    # lands replicated on all partitions (f32 PSUM chunks),
                    # then DVE casts to fp16
                    for c in range(NCH):
                        psB = psu.tile([P, C], f32, tag="ps", name=f"psB_{h}_{c}")
                        nc.tensor.matmul(psB[:], ones1[:],
                                         g_row[:, c * C : (c + 1) * C],
                                         start=True, stop=True)
                        nc.vector.tensor_copy(g_bc[:, c * C : (c + 1) * C],
                                              psB[:])
                elif KNOBS["gbc_dma"] and h not in KNOBS["gbc_pool_heads"]:
                    nc.scalar.dma_start(GROW_d[h : h + 1, :], g_row[:])
                    nc.scalar.dma_start(
                        g_bc[:],
                        GROW_d[h : h + 1, :].partition_broadcast(P).squeeze(1))
                else:
                    nc.gpsimd.partition_broadcast(g_bc[:], g_row[:])

                ss_bc = None
                if S:
                    ss_bc = head.tile([P, S], fp16, tag="ss_bc", bufs=4,
                                      name=f"ss_bc_{h}")
                    nc.scalar.dma_start(SSROW_d[h : h + 1, 0:S], ss_row[:])
                    nc.scalar.dma_start(
                        ss_bc[:],
                        SSROW_d[h : h + 1, 0:S].partition_broadcast(P).squeeze(1))

                G_all = head.tile([P, NT * GW], fp16, tag="G_all", bufs=4,
                                  name=f"G_all_{h}")
                G3 = G_all.rearrange("p (k w) -> p k w", w=GW)
                for halfg in range(2):
                    psG = psu.tile([P, (NT // 2) * FH], f32, tag="ps",
                                   name=f"psG_{h}_{halfg}")
                    for j in range(NT // 2):
                        k = halfg * (NT // 2) + j
                        nc.tensor.matmul(
                            psG[:, j * FH : (j + 1) * FH],
                            XT16[:, k * P : (k + 1) * P],
                            W16[:],
                            start=True, stop=True,
                        )
                    nc.scalar.copy(
                        G3[:, halfg * (NT // 2) : (halfg + 1) * (NT // 2), 0:FH],
                        psG.rearrange("p (k f) -> p k f", f=FH),
                    )
                nc.vector.memset(G3[:, :, FH : FH + 1], 1.0)
                return {"e1g": e1g, "e2g": e2g, "g_bc": g_bc,
                        "G_all": G_all, "agg": None,
                        "snb": snb, "ss_bc": ss_bc}

            def emit_u(h, st, k, use_pool):
                e1g, e2g, g_bc = st["e1g"], st["e2g"], st["g_bc"]
                u_t = stream.tile([P, N], fp16, tag="u", bufs=KNOBS["u_bufs"],
                                  name=f"u_{h}_{k}")
                S = KNOBS["act_split"]
                eng = nc.gpsimd if use_pool else nc.vector
                if S and not use_pool:
                    # columns [0:S] exactly on Act: exp(leaky(ss+sn)); the
                    # per-column softmax factor differs from the DVE half's
                    # convention but cancels in the normalization.
                    tmp = stream.tile([P, S], fp16, tag="uact", bufs=3,
                                      name=f"ua_{h}_{k}")
                    nc.scalar.activation(tmp[:], st["ss_bc"][:], act.Prelu,
                                         bias=st["snb"][:, k : k + 1],
                                         scale=1.0, alpha=alpha02[:])
                    nc.scalar.activation(u_t[:, 0:S], tmp[:], act.Exp)
                    eng.tensor_scalar(
                        u_t[:, S:N], g_bc[:, S:N],
                        e2g[:, k : k + 1], e1g[:, k : k + 1],
                        op.mult, op.max,
                    )
                else:
                    eng.tensor_scalar(
                        u_t[:], g_bc[:],
                        e2g[:, k : k + 1], e1g[:, k : k + 1],
                        op.mult, op.max,
                    )
                return u_t

            def emit_p(h, st, k, u_t, use_pool, tag="p", bufs=None):
                p_t = stream.tile([P, N], fp16, tag=tag,
                                  bufs=bufs or KNOBS["p_bufs"],
                                  name=f"p_{h}_{k}")
                eng = nc.gpsimd if use_pool else nc.vector
                eng.tensor_tensor(
                    p_t[:], u_t[:], AT_sb[:, k * N : (k + 1) * N], op.mult
                )
                return p_t

            def emit_aggs(h, st, k, p_t):
                G_all = st["G_all"]
                if st["agg"] is None:
                    st["agg"] = [
                        psagg.tile([P, g * 65], f32, tag=f"agg{gi}",
                                   name=f"agg{h}_{gi}")
                        for gi, g in enumerate(AGG_GROUPS)
                    ]
                aggs = st["agg"]
                rhs = st["G_all"][:, k * GW : k * GW + 65]
                # PSUM start=True lazily zeroes the whole 2KB bank, so only
                # the FIRST matmul of each bank-group tile may set it; later
                # slices overwrite their pending-zero bytes with start=False.
                for t in range(NT):
                    gi = 0 if t < 7 else (1 if t < 14 else 2)
                    tt_ = t - (0 if t < 7 else (7 if t < 14 else 14))
                    last = AGG_GROUPS[gi] - 1
                    nc.tensor.matmul(
                        aggs[gi][:, tt_ * 65 : tt_ * 65 + 65],
                        p_t[:, t * P : (t + 1) * P],
                        rhs,
                        start=(k == 0 and tt_ == 0),
                        stop=(k == NT - 1 and tt_ == last),
                    )

            def emit_finals(h, st, split_relu=False):
                aggs = st["agg"]
                # den columns (o=64 of each 65-group) -> SBUF, then 1/den
                den = head.tile([P, NT], f32, tag="den", bufs=2,
                                name=f"den_{h}")
                base = 0
                for gi, g in enumerate(AGG_GROUPS):
                    a3 = aggs[gi].rearrange("p (t w) -> p t w", w=65)
                    nc.scalar.copy(den[:, base : base + g], a3[:, :, 64])
                    base += g
                r = head.tile([P, NT], f32, tag="r", bufs=2, name=f"r_{h}")
                nc.vector.reciprocal_approx_fast(r[:], den[:])
                out_sb = outp.tile([P, NT * FH], fp16, tag="outf",
                                   name=f"outf_{h}")
                base = 0
                for gi, g in enumerate(AGG_GROUPS):
                    a3 = aggs[gi].rearrange("p (t w) -> p t w", w=65)
                    for j in range(g):
                        t = base + j
                        if KNOBS["fin_act"] and not (split_relu and t >= KNOBS["split_relu_at"]):
                            nc.scalar.activation(
                                out_sb[:, t * FH : (t + 1) * FH],
                                a3[:, j, 0:FH], act.Relu,
                                scale=r[:, t : t + 1],
                            )
                        else:
                            nc.vector.tensor_scalar(
                                out_sb[:, t * FH : (t + 1) * FH],
                                a3[:, j, 0:FH],
                                r[:, t : t + 1], 0.0, op.mult, op.max,
                            )
                    base += g
                if split_relu:
                    # two half DMAs so the first overlaps the remaining relus
                    ht = NT // 2
                    nc.sync.dma_start(
                        OUT_d[h, 0 : ht * P].rearrange("(t p) f -> p t f", p=P),
                        out_sb.rearrange("p (t f) -> p t f", f=FH)[:, 0:ht, :],
                    )
                    nc.sync.dma_start(
                        OUT_d[h, ht * P : N].rearrange("(t p) f -> p t f", p=P),
                        out_sb.rearrange("p (t f) -> p t f", f=FH)[:, ht:NT, :],
                    )
                else:
                    nc.sync.dma_start(
                        OUT_d[h].rearrange("(t p) f -> p t f", p=P),
                        out_sb.rearrange("p (t f) -> p t f", f=FH),
                    )

            # ---- schedule ------------------------------------------------
            # all four setups run before aggs; heads 0/1 aggregate while A^T
            # streams in, heads 2/3 afterwards.  Pool-assigned u/p ops are
            # emitted with lookahead so the in-order PE agg queue never waits
            # on the slower Pool engine.
            sts = [emit_setup(0), emit_setup(1), None, None]
            seq1 = [(h, k) for k in range(NT) for h in (0, 1)]
            seq2 = [(h, k) for k in range(NT) for h in (2, 3)]

            def pool_set(seq, stride):
                if not stride:
                    return set()
                return {hk for i, hk in enumerate(seq) if i % stride == stride - 1}

            def run_phase(seq, p_pool, u_pool, la, per_k=None, post_k=None,
                          group_done=None):
                pend = {}
                ustore = {}
                emitted = set()
                uahead = KNOBS["u_ahead"]

                def get_u(idx):
                    if idx in ustore:
                        return ustore.pop(idx)
                    h, k = seq[idx]
                    return emit_u(h, sts[h], k, seq[idx] in u_pool)

                def produce_u(idx):
                    # u only needs g_bc: emit ahead so DVE has filler work
                    # while waiting on the A^T merge stream
                    if idx >= len(seq) or idx in ustore or seq[idx] in emitted:
                        return
                    h, k = seq[idx]
                    if sts[h] is None:
                        return
                    ustore[idx] = emit_u(h, sts[h], k, seq[idx] in u_pool)

                def produce(idx):
                    if idx >= len(seq) or seq[idx] in emitted:
                        return
                    h, k = seq[idx]
                    if seq[idx] in p_pool or seq[idx] in u_pool:
                        emitted.add(seq[idx])
                        pend[(h, k)] = emit_p(h, sts[h], k, get_u(idx),
                                              seq[idx] in p_pool)

                lastk = -1
                for i, (h, k) in enumerate(seq):
                    if k != lastk:
                        if per_k is not None:
                            per_k(k)
                        if post_k is not None and lastk >= 0:
                            post_k(lastk)
                        lastk = k
                    for j in range(i, min(i + uahead + 1, len(seq))):
                        produce_u(j)
                    for j in range(i, min(i + la + 1, len(seq))):
                        produce(j)
                    if (h, k) in pend:
                        emit_aggs(h, sts[h], k, pend.pop((h, k)))
                    else:
                        emit_aggs(h, sts[h], k,
                                  emit_p(h, sts[h], k, get_u(i), False))
                    if group_done is not None and (i + 1 == len(seq)
                                                   or seq[i + 1][1] != k):
                        group_done(k)
                if post_k is not None and lastk >= 0:
                    post_k(lastk)

            lead = KNOBS["lead"]

            def per_k1(k):
                if k == 0:
                    emit_merges_to(KNOBS["prologue_merges"])
                if k == KNOBS["setup2_k"]:
                    sts[2] = emit_setup(2)
                if k == KNOBS["setup3_k"]:
                    sts[3] = emit_setup(3)

            def post_k1(k):
                emit_merges_to(k + lead + 2)

            pool1 = pool_set(seq1, KNOBS["p_pool_1"])
            pool2 = pool_set(seq2, KNOBS["p_pool_2"])
            upool1 = pool_set(list(reversed(seq1)), KNOBS["u_pool_1"])
            upool2 = pool_set(list(reversed(seq2)), KNOBS["u_pool_2"])

            # prefill: during phase-1 tail, pre-build head-2 u/p for early k
            # (aggs deferred until its PSUM frees after finals(0)/finals(1))
            prefill = KNOBS["prefill"]
            p2_store = {}

            def group_done1(k):
                j = k - (NT - prefill)
                if 0 <= j < prefill and sts[2] is not None:
                    u2 = emit_u(2, sts[2], j, False)
                    p2_store[j] = emit_p(2, sts[2], j, u2, False,
                                         tag="p2", bufs=max(prefill, 1))

            run_phase(seq1, pool1, upool1, KNOBS["pool_la"], per_k=per_k1,
                      post_k=post_k1, group_done=group_done1)
            emit_finals(0, sts[0])
            emit_finals(1, sts[1])
            seq2a = [(2, k) for k in range(NT) if k not in p2_store]
            seq2b = [(3, k) for k in range(NT)]
            for j in sorted(p2_store):
                emit_aggs(2, sts[2], j, p2_store.pop(j))
            if KNOBS["pool2_split"]:
                # independent pool strides per sub-phase (head 2 vs head 3)
                pool2a = pool_set(seq2a, KNOBS["p_pool_2"])
                pool2b = pool_set(seq2b, KNOBS["p_pool_2b"])
                upool2a = pool_set(list(reversed(seq2a)), KNOBS["u_pool_2"])
                upool2b = pool_set(list(reversed(seq2b)), KNOBS["u_pool_2b"])
            else:
                pool2a = {hk for hk in pool2 if hk[0] == 2 and hk[1] >= prefill}
                pool2b = {hk for hk in pool2 if hk[0] == 3}
                upool2a = {hk for hk in upool2 if hk[0] == 2 and hk[1] >= prefill}
                upool2b = {hk for hk in upool2 if hk[0] == 3}
            run_phase(seq2a, pool2a, upool2a, KNOBS["pool_la"])
            emit_finals(2, sts[2])
            run_phase(seq2b, pool2b, upool2b, KNOBS["pool_la"])
            emit_finals(3, sts[3], split_relu=True)

    nc.compile()
    return nc


def _get_nc():
    if "nc" not in _CACHE:
        _CACHE["nc"] = _build()
    return _CACHE["nc"]


def make_in_maps(inputs):
    X = np.ascontiguousarray(inputs["X"], dtype=np.float32)
    A = np.ascontiguousarray(inputs["A"], dtype=np.float32)
    W = np.ascontiguousarray(inputs["W"], dtype=np.float32)
    b = np.ascontiguousarray(inputs["b"], dtype=np.float32)
    a_self = np.ascontiguousarray(inputs["a_self"], dtype=np.float32)
    a_neigh = np.ascontiguousarray(inputs["a_neigh"], dtype=np.float32)
    # tiny host precompute: WA[h] = [W[h]@a_self[h] | W[h]@a_neigh[h]]
    WA = np.ascontiguousarray(
        np.stack([np.einsum("hfo,ho->hf", W, a_self),
                  np.einsum("hfo,ho->hf", W, a_neigh)], axis=2),
        dtype=np.float32)
    return [
        {
            "A": np.ascontiguousarray(A[i]),
            "X": np.ascontiguousarray(X[i]),
            "W": W,
            "b": b,
            "WA": WA,
        }
        for i in range(B)
    ]


def run(inputs, trace=False):
    from concourse import bass_utils

    nc = _get_nc()
    in_maps = make_in_maps(inputs)
    res = bass_utils.run_bass_kernel_spmd(
        nc, in_maps, core_ids=list(range(B)), trace=trace
    )
    out = np.empty((B, N, H * FH), dtype=np.float32)
    for i in range(B):
        o = np.asarray(res.results[i]["OUT"], dtype=np.float32)  # [H, N, FH]
        out[i] = o.transpose(1, 0, 2).reshape(N, H * FH)
    return out, res


def kernel(**inputs):
    out, _ = run(inputs, trace=False)
    return out
